# revision 1
# baseline (speedup 1.0000x reference)
"""Trainium2 Bass kernel for nn_DynamicFiltering (optimized).

Computation (per batch b):
  y  = LeakyReLU(conv2d(x_t, w1, b1), 0.2)        per frame t
  ker = conv2d(y, w2, b2)                          (t, 9, h, w)
  ker = ker - mean_K(ker) + 1/45                   per-pixel over K = 45
  out[c,h,w] = sum_{t,k1,k2} x_edge[c,t,h+k1-1,w+k2-1] * ker[t,k1,k2][h,w]

Sharding: 8 cores = 2 batches x 4 H-slabs of 32 rows.

Key structure vs the fp32 baseline:
  - all conv matmuls in bf16 (fp32 runs ~2-3 cycles/row on the PE, bf16 1)
  - frames packed in pairs on the 128-partition contraction dim with
    block-diagonal weights: 3 conv passes (f01, f23, f4) instead of 5
  - LeakyReLU = one Act bias-add + one Pool scalar_tensor_tensor max(0.2y,y)
  - conv2 output transposed to pixel-partition layout by the DMA xbar
    (dma_start_transpose), writing [q, ti, r] with r innermost so the
    per-pixel kernel broadcast runs the DVE in 2x bf16 mode
  - dynamic-filter products on DVE as bf16 tensor_tensor (2x), pairwise
    tree accumulation, a few products per pass on the Pool engine
  - the dj column shift of the patches is baked into 3 host-prepared
    shifted copies of x (edge-replicated), so there is a single fp32
    accumulator and no post-transpose merge
  - normalization term: out += c * S with c = 1/45 - mean(ker),
    S = 3x3 box sum of U (U = sum of frames), built from partition-shifted
    copies of U via SBUF-SBUF DMA
"""

import numpy as np

DIM = 64
T = 5
H = 128
W = 128
SLAB = 32          # output rows per core
NCORES = 8
GH = 36            # conv grid rows: slab + 2*2 halo
GW = 130           # conv grid cols: W + 2
FR = 34            # filter rows: slab + 2 halo
NPASS = 3          # frame pairs: (0,1), (2,3), (4,-)

_PROGRAM_CACHE = {}


def _build_program():
    import concourse.bacc as bacc
    import concourse.mybir as mybir
    from concourse.tile import TileContext

    f32 = mybir.dt.float32
    bf16 = mybir.dt.bfloat16
    u16 = mybir.dt.uint16
    Act = mybir.ActivationFunctionType
    Alu = mybir.AluOpType
    Ax = mybir.AxisListType

    nc = bacc.Bacc("TRN2", debug=False)

    xc_d = nc.dram_tensor("xc", [NPASS, 128, GH, GW], bf16, kind="ExternalInput").ap()
    xt_d = nc.dram_tensor("xt", [3, W, T, DIM, FR], bf16, kind="ExternalInput").ap()
    w1a_d = nc.dram_tensor("w1a", [128, 9, 128], bf16, kind="ExternalInput").ap()
    w1b_d = nc.dram_tensor("w1b", [128, 9, 128], bf16, kind="ExternalInput").ap()
    w2a_d = nc.dram_tensor("w2a", [128, 9, 18], bf16, kind="ExternalInput").ap()
    w2b_d = nc.dram_tensor("w2b", [128, 9, 18], bf16, kind="ExternalInput").ap()
    b1_d = nc.dram_tensor("b1r", [128, 1], f32, kind="ExternalInput").ap()
    b1s_d = nc.dram_tensor("b1s", [128, 1], f32, kind="ExternalInput").ap()
    b2a_d = nc.dram_tensor("b2a", [18, 1], f32, kind="ExternalInput").ap()
    b2b_d = nc.dram_tensor("b2b", [18, 1], f32, kind="ExternalInput").ap()
    ym_d = nc.dram_tensor("ymask", [128, 2], f32, kind="ExternalInput").ap()
    id_d = nc.dram_tensor("ident", [128, 128], f32, kind="ExternalInput").ap()
    idb_d = nc.dram_tensor("identb", [128, 128], bf16, kind="ExternalInput").ap()
    out_d = nc.dram_tensor("out", [DIM, SLAB, W], f32, kind="ExternalOutput").ap()

    # conv1 chunks: grid out rows 1..34;  conv2 chunks: grid out rows 2..33
    # (4 rows per chunk: a matmul's PSUM output must fit one 2KB bank)
    C1_CHUNKS = [(1 + 4 * i, 4) for i in range(8)] + [(33, 2)]
    C2_CHUNKS = [(2 + 4 * i, 4) for i in range(8)]

    with TileContext(nc) as tc:
        with (
            tc.tile_pool(name="consts", bufs=1) as cpool,
            tc.tile_pool(name="xtp", bufs=1) as xtp,
            tc.tile_pool(name="xcp", bufs=2) as xcp,
            tc.tile_pool(name="yp", bufs=2) as yp,
            tc.tile_pool(name="y0p", bufs=2) as y0p,
            tc.tile_pool(name="kst", bufs=1) as kstp,
            tc.tile_pool(name="ktp", bufs=1) as ktp,
            tc.tile_pool(name="up", bufs=1) as up,
            tc.tile_pool(name="tp", bufs=7) as tp,
            tc.tile_pool(name="obp", bufs=3) as obp,
        ):
            # conv input in 5 row-band sub-DMAs into one tile: conv chunk b
            # then only waits for the bands covering its rows.  Pass-0's
            # bands are THE FIRST descriptors issued (the DMA engines drain
            # roughly in arrival order, so anything queued ahead of them —
            # weights or the 8.4MB filter input — delays the first matmul).
            def load_xc(p):
                t = xcp.tile([128, GH, GW], bf16, tag="xc")
                for r0b, r1b in ((0, 8), (8, 16), (16, 24), (24, 32), (32, 36)):
                    nc.sync.dma_start(out=t[:, r0b:r1b],
                                      in_=xc_d[p, :, r0b:r1b])
                return t

            xc_next = load_xc(0)

            w1a_sb = cpool.tile([128, 9, 128], bf16)
            nc.sync.dma_start(out=w1a_sb, in_=w1a_d)
            b1_sb = cpool.tile([128, 1], f32)
            nc.sync.dma_start(out=b1_sb, in_=b1_d)
            b1s_sb = cpool.tile([128, 1], f32)
            nc.sync.dma_start(out=b1s_sb, in_=b1s_d)
            w1b_sb = cpool.tile([128, 9, 128], bf16)
            nc.sync.dma_start(out=w1b_sb, in_=w1b_d)
            w2a_sb = cpool.tile([128, 9, 18], bf16)
            nc.sync.dma_start(out=w2a_sb, in_=w2a_d)
            w2b_sb = cpool.tile([128, 9, 18], bf16)
            nc.sync.dma_start(out=w2b_sb, in_=w2b_d)
            b2a_sb = cpool.tile([18, 1], f32)
            nc.sync.dma_start(out=b2a_sb, in_=b2a_d)
            b2b_sb = cpool.tile([18, 1], f32)
            nc.sync.dma_start(out=b2b_sb, in_=b2b_d)
            ym_sb = cpool.tile([128, 2], f32)
            nc.sync.dma_start(out=ym_sb, in_=ym_d)
            id_sb = cpool.tile([128, 128], f32)
            nc.sync.dma_start(out=id_sb, in_=id_d)
            idb_sb = cpool.tile([128, 128], bf16)
            nc.sync.dma_start(out=idb_sb, in_=idb_d)

            # conv2 -> kernel staging (bf16, pixel cols innermost) and the
            # transposed per-pixel kernels kt2[p][q, ti, r] (r innermost).
            # The memset runs first on the gpsimd queue: its ~3.4us delay the
            # xt DMA issue below past the band/weight issues on sync.
            ker_st = kstp.tile([32, SLAB, W], bf16)
            nc.gpsimd.memset(ker_st.bitcast(u16), 0)

            # shifted x copies for the filter: xt[dj][q] = x[.., q + dj - 1]
            # (edge-replicated at q=0/127 by the host)
            xt = [xtp.tile([W, T, DIM, FR], bf16, name=f"xt{dj}")
                  for dj in range(3)]
            for dj in range(3):
                nc.gpsimd.dma_start(out=xt[dj], in_=xt_d[dj])
            kt2 = [ktp.tile([W, 32, SLAB], bf16, name=f"kt2_{p}")
                   for p in range(NPASS)]
            ktap = kstp  # transpose staging shares the kst pool
            pass_sums = [ktp.tile([W, DIM, SLAB], bf16, name=f"psum{p}")
                         for p in range(NPASS)]

            # --- S path: U = sum_t x_t, U3 = sum_dj U_dj, Sv = 3-row box ---
            u_c = up.tile([W, DIM, FR], bf16, name="u_c")
            u_m1 = up.tile([W, DIM, FR], bf16, name="u_m1")
            u_p1 = up.tile([W, DIM, FR], bf16, name="u_p1")
            sv = up.tile([W, DIM, SLAB], bf16, name="sv")

            # U chain part 1 on DVE (idle until the first products anyway);
            # the slow single-engine SBUF-SBUF shift DMAs are issued early so
            # they finish long before Sv is needed at the tail
            nc.vector.tensor_tensor(u_c, xt[1][:, 0], xt[1][:, 1], Alu.add)
            for t_i in (2, 3, 4):
                nc.vector.tensor_tensor(u_c, u_c, xt[1][:, t_i], Alu.add)
            # partition-shifted copies (DMA is exempt from the engine
            # start-partition restriction); edges replicate
            nc.gpsimd.dma_start(out=u_m1[1:128], in_=u_c[0:127])
            nc.gpsimd.dma_start(out=u_m1[0:1], in_=u_c[0:1])
            nc.gpsimd.dma_start(out=u_p1[0:127], in_=u_c[1:128])
            nc.gpsimd.dma_start(out=u_p1[127:128], in_=u_c[127:128])

            def emit_u_part2():
                # U3/Sv only feed c*S at the tail: emitted post-loop so they
                # never sit in the DVE queue ahead of the pass-0 products
                nc.vector.tensor_tensor(u_m1, u_c, u_m1, Alu.add)
                nc.vector.tensor_tensor(u_m1, u_m1, u_p1, Alu.add)
                nc.vector.tensor_tensor(sv, u_m1[:, :, 0:SLAB],
                                        u_m1[:, :, 1:SLAB + 1], Alu.add)
                nc.vector.tensor_tensor(sv, sv, u_m1[:, :, 2:SLAB + 2], Alu.add)

            with (
                tc.tile_pool(name="ps1", bufs=4, space="PSUM") as ps1p,
                tc.tile_pool(name="ps2", bufs=3, space="PSUM") as ps2p,
            ):
                for p in range(NPASS):
                    w1_sb = w1a_sb if p < 2 else w1b_sb
                    w2_sb = w2a_sb if p < 2 else w2b_sb
                    b2_sb = b2a_sb if p < 2 else b2b_sb

                    # prefetch the next pass's bands now: the sync queue
                    # blocks at this pass's dma_start_transpose until conv2
                    # finishes, which would delay anything emitted after it
                    xc_f = xc_next
                    if p + 1 < NPASS:
                        xc_next = load_xc(p + 1)
                    y_f = yp.tile([128, GH, GW], bf16, tag="y")
                    nc.gpsimd.memset(y_f[:, 1:35, 0:1].bitcast(u16), 0)
                    nc.gpsimd.memset(y_f[:, 1:35, 129:130].bitcast(u16), 0)

                    # conv1 + leaky relu
                    for ci, (g0, nr) in enumerate(C1_CHUNKS):
                        ps = ps1p.tile([128, 4, W], f32, tag="ps1")
                        for idx in range(9):
                            di, dj = divmod(idx, 3)
                            rhs = xc_f[:, g0 + di - 1:g0 + di - 1 + nr, dj:dj + W]
                            nc.tensor.matmul(
                                ps[:, :nr, :],
                                lhsT=w1_sb[:, idx, :],
                                rhs=rhs,
                                start=(idx == 0),
                                stop=(idx == 8),
                            )
                        y0 = y0p.tile([128, 4, W], bf16, tag="y0")
                        a0 = y0p.tile([128, 4, W], bf16, tag="a0")
                        nc.scalar.activation(y0[:, :nr], ps[:, :nr], Act.Identity,
                                             bias=b1_sb, scale=1.0)
                        # a0 = (2/3)|y0| via Abs((2/3) ps + (2/3) b1)
                        nc.scalar.activation(a0[:, :nr], ps[:, :nr], Act.Abs,
                                             bias=b1s_sb, scale=2.0 / 3.0)
                        # y = y0 + (2/3)|y0| == leaky(y0)/0.6; w2 carries the
                        # 0.6.  On Pool: DVE must stay free for the products.
                        nc.gpsimd.tensor_tensor(y_f[:, g0:g0 + nr, 1:129],
                                                y0[:, :nr], a0[:, :nr], Alu.add)
                    # conv2 zero-pads rows outside the image: kill y halo rows
                    nc.scalar.activation(y_f[:, 1:2, 1:129], y_f[:, 1:2, 1:129],
                                         Act.Copy, scale=ym_sb[:, 0:1])
                    nc.scalar.activation(y_f[:, 34:35, 1:129], y_f[:, 34:35, 1:129],
                                         Act.Copy, scale=ym_sb[:, 1:2])

                    # conv2 -> ker_st[ti, r, q]; grid row = 2 + r
                    for g0, nr in C2_CHUNKS:
                        ps2 = ps2p.tile([18, 4, W], f32, tag="ps2")
                        for idx in range(9):
                            di, dj = divmod(idx, 3)
                            rhs = y_f[:, g0 + di - 1:g0 + di - 1 + nr, dj:dj + W]
                            nc.tensor.matmul(
                                ps2,
                                lhsT=w2_sb[:, idx, :],
                                rhs=rhs,
                                start=(idx == 0),
                                stop=(idx == 8),
                            )
                        nc.scalar.activation(ker_st[0:18, g0 - 2:g0 - 2 + nr, :],
                                             ps2, Act.Identity, bias=b2_sb,
                                             scale=1.0)

                    # DMA xbar transpose: ker_st[ti, (r, q)] -> ktA[q, r, ti]
                    # (DMA needs a contiguous last dim), then DVE repack to
                    # kt2[q, ti, r] so the product broadcast has r innermost
                    ktA = ktap.tile([W, SLAB, 32], bf16, tag="ktA")
                    nc.sync.dma_start_transpose(
                        out=ktA, in_=ker_st.rearrange("ti r q -> ti (r q)"))
                    nc.vector.tensor_copy(
                        kt2[p], ktA.rearrange("q r ti -> q ti r"))

                    # dynamic-filter products for this pass's frames
                    frames = [2 * p, 2 * p + 1] if p < 2 else [4]
                    terms = [(t_i, fi, di, dj)
                             for fi, t_i in enumerate(frames)
                             for di in range(3) for dj in range(3)]
                    n_pool = 0 if p < 2 else 3
                    stack = []
                    for i, (t_i, fi, di, dj) in enumerate(terms):
                        ti = fi * 9 + 3 * di + dj
                        kb = kt2[p][:, ti, :].unsqueeze(1)\
                            .broadcast_to((W, DIM, SLAB))
                        xs = xt[dj][:, t_i, :, di:di + SLAB]
                        prod = tp.tile([W, DIM, SLAB], bf16, tag="ts")
                        eng = nc.gpsimd if i >= len(terms) - n_pool else nc.vector
                        eng.tensor_tensor(prod, xs, kb, Alu.mult)
                        cur, sz = prod, 1
                        while stack and stack[-1][1] == sz:
                            prev, _ = stack.pop()
                            nxt = tp.tile([W, DIM, SLAB], bf16, tag="ts")
                            nc.vector.tensor_tensor(nxt, prev, cur, Alu.add)
                            cur, sz = nxt, sz * 2
                        stack.append((cur, sz))
                    while len(stack) > 2:
                        b_, _ = stack.pop()
                        a_, _ = stack.pop()
                        nxt = tp.tile([W, DIM, SLAB], bf16, tag="ts")
                        nc.vector.tensor_tensor(nxt, a_, b_, Alu.add)
                        stack.append((nxt, 0))
                    b_, _ = stack.pop()
                    a_, _ = stack.pop()
                    nc.vector.tensor_tensor(pass_sums[p], a_, b_, Alu.add)

            emit_u_part2()

            # --- normalization coefficient c = 1/45 - mean(ker) ---
            r0_ = ktp.tile([W, SLAB], f32, name="r0")
            r1_ = ktp.tile([W, SLAB], f32, name="r1")
            r2_ = ktp.tile([W, SLAB], f32, name="r2")
            nc.vector.tensor_reduce(r0_, kt2[0].rearrange("q ti r -> q r ti")
                                    [:, :, 0:18], axis=Ax.X, op=Alu.add)
            nc.vector.tensor_reduce(r1_, kt2[1].rearrange("q ti r -> q r ti")
                                    [:, :, 0:18], axis=Ax.X, op=Alu.add)
            nc.vector.tensor_reduce(r2_, kt2[2].rearrange("q ti r -> q r ti")
                                    [:, :, 0:9], axis=Ax.X, op=Alu.add)
            nc.vector.tensor_tensor(r0_, r0_, r1_, Alu.add)
            nc.vector.tensor_tensor(r0_, r0_, r2_, Alu.add)
            c_sb = ktp.tile([W, SLAB], f32, name="c_sb")
            nc.vector.tensor_scalar(c_sb, r0_, -1.0 / 45.0, 1.0 / 45.0,
                                    Alu.mult, Alu.add)
            c_bf = ktp.tile([W, SLAB], bf16, name="c_bf")
            nc.vector.tensor_copy(c_bf, c_sb)

            # out += c * S  (joins the final combine)
            cs_prod = tp.tile([W, DIM, SLAB], bf16, tag="ts")
            cb = c_bf.unsqueeze(1).broadcast_to((W, DIM, SLAB))
            nc.vector.tensor_tensor(cs_prod, sv, cb, Alu.mult)

            # final combine on the PE (idle by now): fp32 PSUM accumulation
            # of the three pass sums and c*S via bf16-identity matmuls, then
            # per-512-chunk Act drain + output transposes
            total = ktp.tile([W, DIM, SLAB], f32, name="total")
            tot_flat = total.rearrange("q c r -> q (c r)")
            out_rc = out_d.rearrange("c r w -> (c r) w")
            tiles = [pass_sums[0], pass_sums[1], pass_sums[2], cs_prod]
            with (
                tc.tile_pool(name="acc", bufs=1, space="PSUM") as accp,
                tc.tile_pool(name="pso", bufs=2, space="PSUM") as psop,
            ):
                acc = accp.tile([W, DIM * SLAB], f32)
                for ti_i, tl in enumerate(tiles):
                    fl = tl.rearrange("q c r -> q (c r)")
                    for cc in range(4):
                        sl = slice(512 * cc, 512 * (cc + 1))
                        nc.tensor.matmul(
                            acc[:, sl], lhsT=idb_sb, rhs=fl[:, sl],
                            start=(ti_i == 0), stop=(ti_i == len(tiles) - 1))
                for cc in range(4):
                    sl = slice(512 * cc, 512 * (cc + 1))
                    nc.scalar.activation(tot_flat[:, sl], acc[:, sl],
                                         Act.Copy, scale=1.0)
                    for oc in range(4 * cc, 4 * cc + 4):
                        so = slice(128 * oc, 128 * (oc + 1))
                        pst = psop.tile([128, 128], f32, tag="pso")
                        nc.tensor.transpose(pst, tot_flat[:, so], id_sb)
                        ob = obp.tile([128, 128], f32, tag="ob")
                        nc.scalar.activation(ob, pst, Act.Copy, scale=1.0)
                        nc.sync.dma_start(out=out_rc[so], in_=ob)

    return nc


def _get_program():
    if "nc" not in _PROGRAM_CACHE:
        nc = _build_program()
        nc.finalize()
        _PROGRAM_CACHE["nc"] = nc
    return _PROGRAM_CACHE["nc"]


def _host_prep(x, w1, b1, w2, b2):
    """Build the 8 per-core input maps from full inputs."""
    import ml_dtypes
    bf = ml_dtypes.bfloat16

    x = np.asarray(x, dtype=np.float32)
    w1 = np.asarray(w1, dtype=np.float32)
    b1 = np.asarray(b1, dtype=np.float32)
    w2 = np.asarray(w2, dtype=np.float32)
    b2 = np.asarray(b2, dtype=np.float32)

    # block-diagonal packed weights (2 frames per conv pass)
    w1t = w1.transpose(1, 2, 3, 0).reshape(DIM, 9, DIM)  # [ci, tap, o]
    # y is stored as leaky/0.6 (the Abs trick); fold the 0.6 into w2
    w2t = 0.6 * w2.transpose(1, 2, 3, 0).reshape(DIM, 9, 9)
    w1a = np.zeros((128, 9, 128), np.float32)
    w1a[0:64, :, 0:64] = w1t
    w1a[64:128, :, 64:128] = w1t
    w1b = np.zeros((128, 9, 128), np.float32)
    w1b[0:64, :, 0:64] = w1t
    w2a = np.zeros((128, 9, 18), np.float32)
    w2a[0:64, :, 0:9] = w2t
    w2a[64:128, :, 9:18] = w2t
    w2b = np.zeros((128, 9, 18), np.float32)
    w2b[0:64, :, 0:9] = w2t

    b1r = np.concatenate([b1, b1]).reshape(128, 1).astype(np.float32)
    b1s = ((2.0 / 3.0) * b1r).astype(np.float32)
    b2a = np.concatenate([b2, b2]).reshape(18, 1).astype(np.float32)
    b2b = np.concatenate([b2, 0 * b2]).reshape(18, 1).astype(np.float32)
    ident = np.eye(128, dtype=np.float32)
    identb = np.eye(128, dtype=np.float32).astype(bf)

    w1a = w1a.astype(bf)
    w1b = w1b.astype(bf)
    w2a = w2a.astype(bf)
    w2b = w2b.astype(bf)

    in_maps = []
    for core in range(NCORES):
        b, s = divmod(core, 4)
        r0 = s * SLAB
        # conv input per pass: frames (2p, 2p+1) stacked on 128 partitions,
        # rows r0-2 .. r0+33 zero padded, cols -1..128 zero padded
        xc = np.zeros((NPASS, 128, GH, GW), np.float32)
        lo = max(0, r0 - 2)
        hi = min(H, r0 + 34)
        for p in range(NPASS):
            for f in range(2):
                t = 2 * p + f
                if t >= T:
                    continue
                xc[p, f * 64:(f + 1) * 64,
                   lo - (r0 - 2):hi - (r0 - 2), 1:129] = x[b, :, t, lo:hi, :]
        # filter input, pixel-partition, 3 dj-shifted copies:
        # xt[dj][q, t, c, rr] = x[b, c, t, clip(r0-1+rr), clip(q+dj-1)]
        rows = np.clip(np.arange(r0 - 1, r0 + 33), 0, H - 1)
        base = x[b][:, :, rows, :]            # (c, t, 34, w)
        xt = np.empty((3, W, T, DIM, FR), np.float32)
        for dj in range(3):
            cols = np.clip(np.arange(dj - 1, W + dj - 1), 0, W - 1)
            xt[dj] = base[:, :, :, cols].transpose(3, 1, 0, 2)
        # conv2 zero-pad mask for the y halo rows (grid rows 1 and 34)
        ymask = np.ones((128, 2), np.float32)
        if s == 0:
            ymask[:, 0] = 0.0
        if s == 3:
            ymask[:, 1] = 0.0
        in_maps.append({
            "xc": xc.astype(bf), "xt": xt.astype(bf),
            "w1a": w1a, "w1b": w1b, "w2a": w2a, "w2b": w2b,
            "b1r": b1r, "b1s": b1s, "b2a": b2a, "b2b": b2b,
            "ymask": ymask, "ident": ident, "identb": identb,
        })
    return in_maps


def kernel(x, w1, b1, w2, b2):
    from concourse.bass_utils import run_bass_kernel_spmd

    nc = _get_program()
    in_maps = _host_prep(x, w1, b1, w2, b2)
    res = run_bass_kernel_spmd(nc, in_maps, list(range(NCORES)))
    out = np.zeros((2, DIM, H, W), dtype=np.float32)
    for core in range(NCORES):
        b, s = divmod(core, 4)
        out[b, :, s * SLAB:(s + 1) * SLAB, :] = res.results[core]["out"]
    return out



# revision 19
# speedup vs baseline: 1.0308x; 1.0308x over previous
"""Trainium2 Bass kernel for nn_DynamicFiltering (v2).

Computation (per batch b):
  y  = LeakyReLU(conv2d(x_t, w1, b1), 0.2)        per frame t
  ker = conv2d(y, w2, b2)                          (t, 9, h, w)
  ker = ker - mean_K(ker) + 1/45                   per-pixel over K = 45
  out[c,h,w] = sum_{t,k1,k2} x_edge[c,t,h+k1-1,w+k2-1] * ker[t,k1,k2][h,w]

Sharding: 8 cores = 2 batches x 4 H-slabs of 32 rows.

Structure (v2, vs the 234us bf16 baseline):
  - all 16-bit data is fp16 (same speed as bf16, ~8x less quant error)
  - conv1 leaky relu is a single Act Prelu(alpha=0.2) drain (verified on
    HW: Prelu honors alpha, Lrelu hardcodes 0.01) -> GpSimd fully freed
  - pass 2 (lone frame 4) is spatially halved: both partition halves
    carry frame-4 channels over half the rows, same block-diag weights
    as the 2-frame passes -> conv1/conv2 pass-2 matmul cycles halve
  - dynamic-filter products run on DVE in 2x fp16 mode: the di=1 row
    window is served by a separate host copy (xts) so every slice start
    is 4B-aligned
  - only the CENTER (dj=1) pixel-partition x copy is loaded; the dj
    column shift moves to the kernel side: kt2 is partition-shifted by
    +-1 via SBUF-SBUF DMA (2KB/partition vs 40KB for x copies) and the
    PE accumulate uses shifted identity matrices (eye(k=+-1)).  The
    edge-replication terms (q=0 dj=0, q=127 dj=2) multiply the same x
    element as the dj=1 term, so they fold into the dj=1 kernel's edge
    values with one tiny DVE add per edge per pass
  - no tree reduction / pass sums: product pairs (same dj) are added
    once on DVE, then PE matmuls accumulate each pair tile into a
    persistent 4-bank PSUM accumulator, interleaved with the next
    pass's conv matmuls (PSUM: 2 conv1 + 2 conv2 + 4 acc banks = 8)
  - normalization term c*S as in v1 (U chain: sums on DVE in the
    pre-product idle window, shifts/box-sums on GpSimd)
  - output: acc -> fp16 -> 16 DMA-xbar transposes -> DRAM (host casts
    to fp32); no PE transposes, no fp32 identity
  - startup: w1 + pass-0 conv input bands issued first on the sync
    HWDGE ring; all other constants + pass-1/2 inputs on the act ring
"""

import numpy as np

DIM = 64
T = 5
H = 128
W = 128
SLAB = 32          # output rows per core
NCORES = 8
GH = 36            # conv grid rows, passes 0/1: slab + 2*2 halo
GH2 = 20           # conv grid rows, pass 2 halves: 16 + 2*2 halo
GW = 130           # conv grid cols: W + 2
FR = 34            # filter rows: slab + 2 halo
NPASS = 3

_PROGRAM_CACHE = {}

C1_CHUNKS = [(1 + 4 * i, 4) for i in range(8)] + [(33, 2)]
C2_CHUNKS = [(2 + 4 * i, 4) for i in range(8)]
C1_CHUNKS2 = [(1 + 4 * i, 4) for i in range(4)] + [(17, 2)]
C2_CHUNKS2 = [(2 + 4 * i, 4) for i in range(4)]


def _build_program():
    import concourse.bacc as bacc
    import concourse.mybir as mybir
    from concourse.tile import TileContext

    f32 = mybir.dt.float32
    f16 = mybir.dt.float16
    u16 = mybir.dt.uint16
    Act = mybir.ActivationFunctionType
    Alu = mybir.AluOpType
    Ax = mybir.AxisListType

    nc = bacc.Bacc("TRN2", debug=False)

    xc01_d = nc.dram_tensor("xc01", [2, 128, GH, GW], f16, kind="ExternalInput").ap()
    xc2_d = nc.dram_tensor("xc2", [128, GH2, GW], f16, kind="ExternalInput").ap()
    xt_d = nc.dram_tensor("xt", [W, T, DIM, FR], f16, kind="ExternalInput").ap()
    xts_d = nc.dram_tensor("xts", [W, T, DIM, SLAB], f16, kind="ExternalInput").ap()
    sm_d = nc.dram_tensor("sm", [128, 2, 128], f16, kind="ExternalInput").ap()
    em_d = nc.dram_tensor("em", [128, 2], f32, kind="ExternalInput").ap()
    w1_d = nc.dram_tensor("w1", [128, 9, 128], f16, kind="ExternalInput").ap()
    w2_d = nc.dram_tensor("w2", [128, 9, 18], f16, kind="ExternalInput").ap()
    b1_d = nc.dram_tensor("b1r", [128, 1], f32, kind="ExternalInput").ap()
    b2_d = nc.dram_tensor("b2r", [18, 1], f32, kind="ExternalInput").ap()
    ym_d = nc.dram_tensor("ym", [128, 2], f32, kind="ExternalInput").ap()
    ym2_d = nc.dram_tensor("ym2", [128, 2], f32, kind="ExternalInput").ap()
    idf_d = nc.dram_tensor("idf", [128, 128], f16, kind="ExternalInput").ap()
    out_d = nc.dram_tensor("out", [DIM, SLAB, W], f16, kind="ExternalOutput").ap()

    with TileContext(nc) as tc:
        with (
            tc.tile_pool(name="consts", bufs=1) as cpool,
            tc.tile_pool(name="xtp", bufs=1) as xtp,
            tc.tile_pool(name="xcp", bufs=2) as xcp,
            tc.tile_pool(name="xc2p", bufs=1) as xc2p,
            tc.tile_pool(name="yp", bufs=3) as yp,
            tc.tile_pool(name="ksh", bufs=2) as kshp,
            tc.tile_pool(name="kst", bufs=1) as kstp,
            tc.tile_pool(name="kta", bufs=1) as ktap,
            tc.tile_pool(name="ktp", bufs=1) as ktp,
            tc.tile_pool(name="up", bufs=1) as up,
            tc.tile_pool(name="tp", bufs=8) as tp,
            tc.tile_pool(name="obp", bufs=4) as obp,
        ):
            # ---- startup DMAs: sync ring carries only what gates the ----
            # ---- first conv1 matmuls (w1 + pass-0 input bands)        ----
            w1_sb = cpool.tile([128, 9, 128], f16)
            nc.sync.dma_start(out=w1_sb, in_=w1_d)

            def load_xc01(p, eng):
                t = xcp.tile([128, GH, GW], f16, tag="xc")
                for r0b, r1b in ((0, 8), (8, 16), (16, 24), (24, 32), (32, 36)):
                    eng.dma_start(out=t[:, r0b:r1b], in_=xc01_d[p, :, r0b:r1b])
                return t

            xc_p0 = load_xc01(0, nc.sync)

            # act HWDGE ring: everything else, in need-order
            b1_sb = cpool.tile([128, 1], f32)
            nc.scalar.dma_start(out=b1_sb, in_=b1_d)
            w2_sb = cpool.tile([128, 9, 18], f16)
            nc.scalar.dma_start(out=w2_sb, in_=w2_d)
            b2_sb = cpool.tile([18, 1], f32)
            nc.scalar.dma_start(out=b2_sb, in_=b2_d)
            ym_sb = cpool.tile([128, 2], f32)
            nc.scalar.dma_start(out=ym_sb, in_=ym_d)
            ym2_sb = cpool.tile([128, 2], f32)
            nc.scalar.dma_start(out=ym2_sb, in_=ym2_d)
            idf_sb = cpool.tile([128, 128], f16)
            nc.scalar.dma_start(out=idf_sb, in_=idf_d)
            sm_sb = cpool.tile([128, 2, 128], f16)
            nc.scalar.dma_start(out=sm_sb, in_=sm_d)
            em_sb = cpool.tile([128, 2], f32)
            nc.scalar.dma_start(out=em_sb, in_=em_d)
            xc_p1 = load_xc01(1, nc.scalar)
            xc_p2 = xc2p.tile([128, GH2, GW], f16)
            for r0b, r1b in ((0, 8), (8, 16), (16, 20)):
                nc.scalar.dma_start(out=xc_p2[:, r0b:r1b], in_=xc2_d[:, r0b:r1b])

            # conv2 -> kernel staging (ti on partitions); the memset also
            # delays the big xt issues below past w1/xc band issues
            ker_st = kstp.tile([32, SLAB, W], f16)
            nc.gpsimd.memset(ker_st.bitcast(u16), 0)

            xt1 = xtp.tile([W, T, DIM, FR], f16, name="xt1")
            xts1 = xtp.tile([W, T, DIM, SLAB], f16, name="xts1")
            nc.gpsimd.dma_start(out=xt1, in_=xt_d)
            nc.gpsimd.dma_start(out=xts1, in_=xts_d)

            # y tiles pre-allocated; edge cols zeroed up front on gpsimd
            y_t = [yp.tile([128, GH, GW], f16, name=f"y{p}", tag="y")
                   for p in range(3)]
            for p in range(3):
                nr = 34 if p < 2 else 18
                nc.gpsimd.memset(y_t[p][:, 1:1 + nr, 0:1].bitcast(u16), 0)
                nc.gpsimd.memset(y_t[p][:, 1:1 + nr, 129:130].bitcast(u16), 0)

            kt2 = [ktp.tile([W, 32, SLAB], f16, name=f"kt2_{p}")
                   for p in range(NPASS)]

            # U chain part 1 on DVE: fills the idle window before the
            # first products (xt arrives ~35us, first kernels ~41us)
            u_c = up.tile([W, DIM, FR], f16, name="u_c")
            u_m1 = up.tile([W, DIM, FR], f16, name="u_m1")
            u_p1 = up.tile([W, DIM, FR], f16, name="u_p1")
            sv = up.tile([W, DIM, SLAB], f16, name="sv")
            nc.vector.tensor_tensor(u_c, xt1[:, 0], xt1[:, 1], Alu.add)
            for t_i in (2, 3, 4):
                nc.vector.tensor_tensor(u_c, u_c, xt1[:, t_i], Alu.add)

            def emit_u_part2():
                # partition-shifted copies via DMA (engines are lockstep;
                # only DMA can shift partitions); edges replicate
                nc.gpsimd.dma_start(out=u_m1[1:128], in_=u_c[0:127])
                nc.gpsimd.dma_start(out=u_m1[0:1], in_=u_c[0:1])
                nc.gpsimd.dma_start(out=u_p1[0:127], in_=u_c[1:128])
                nc.gpsimd.dma_start(out=u_p1[127:128], in_=u_c[127:128])
                nc.gpsimd.tensor_tensor(u_m1, u_c, u_m1, Alu.add)
                nc.gpsimd.tensor_tensor(u_m1, u_m1, u_p1, Alu.add)
                nc.gpsimd.tensor_tensor(sv, u_m1[:, :, 0:SLAB],
                                        u_m1[:, :, 1:SLAB + 1], Alu.add)
                nc.gpsimd.tensor_tensor(sv, sv, u_m1[:, :, 2:SLAB + 2], Alu.add)

            r_p = [ktp.tile([W, SLAB], f32, name=f"r{p}") for p in range(NPASS)]

            with (
                tc.tile_pool(name="ps1", bufs=2, space="PSUM") as ps1p,
                tc.tile_pool(name="ps2", bufs=2, space="PSUM") as ps2p,
                tc.tile_pool(name="acc", bufs=1, space="PSUM") as accp,
            ):
                acc = accp.tile([W, DIM * SLAB], f32)
                pending = []          # (tile, dj) awaiting PE accumulate
                acc_state = {"first": True}

                def acc_mm(tile, dj, last):
                    # dj=1: plain identity; dj=0/2: shifted identity
                    # applies the +-1 pixel-column shift of the patches
                    lhs = (sm_sb[:, 0, :], idf_sb, sm_sb[:, 1, :])[dj]
                    fl = tile.rearrange("q c r -> q (c r)")
                    for cc in range(4):
                        sl = slice(512 * cc, 512 * (cc + 1))
                        nc.tensor.matmul(acc[:, sl], lhsT=lhs, rhs=fl[:, sl],
                                         start=acc_state["first"], stop=last)
                    acc_state["first"] = False

                def drain_acc(n):
                    for _ in range(min(n, len(pending))):
                        tile, dj = pending.pop(0)
                        acc_mm(tile, dj, False)

                def final_drain():
                    while len(pending) > 1:
                        drain_acc(1)
                    tile, dj = pending.pop(0)
                    acc_mm(tile, dj, True)

                for p in range(NPASS):
                    xc_f = (xc_p0, xc_p1, xc_p2)[p]
                    y_f = y_t[p]
                    c1 = C1_CHUNKS if p < 2 else C1_CHUNKS2
                    c2 = C2_CHUNKS if p < 2 else C2_CHUNKS2

                    # conv1 + leaky relu (single Prelu drain per chunk)
                    for ci, (g0, nr) in enumerate(c1):
                        ps = ps1p.tile([128, 4, W], f32, tag="ps1")
                        for idx in range(9):
                            di, dj = divmod(idx, 3)
                            rhs = xc_f[:, g0 + di - 1:g0 + di - 1 + nr,
                                       dj:dj + W]
                            nc.tensor.matmul(
                                ps[:, :nr, :], lhsT=w1_sb[:, idx, :], rhs=rhs,
                                start=(idx == 0), stop=(idx == 8))
                        nc.scalar.activation(y_f[:, g0:g0 + nr, 1:129],
                                             ps[:, :nr], Act.Prelu,
                                             bias=b1_sb, scale=1.0, alpha=0.2)
                        # interleave pending accumulates from the previous
                        # pass (PE slightly ahead of DVE -> no stall)
                        if p > 0 and ci >= 2 and ci % 2 == 0:
                            drain_acc(1)

                    # conv2 zero-pads rows outside the image: kill y halo
                    ymm = ym_sb if p < 2 else ym2_sb
                    hrow = 34 if p < 2 else 18
                    nc.scalar.activation(y_f[:, 1:2, 1:129],
                                         y_f[:, 1:2, 1:129],
                                         Act.Copy, scale=ymm[:, 0:1])
                    nc.scalar.activation(y_f[:, hrow:hrow + 1, 1:129],
                                         y_f[:, hrow:hrow + 1, 1:129],
                                         Act.Copy, scale=ymm[:, 1:2])

                    # conv2 -> ker_st[ti, r, q]
                    for ci, (g0, nr) in enumerate(c2):
                        ps2 = ps2p.tile([18, 4, W], f32, tag="ps2")
                        for idx in range(9):
                            di, dj = divmod(idx, 3)
                            rhs = y_f[:, g0 + di - 1:g0 + di - 1 + nr,
                                      dj:dj + W]
                            nc.tensor.matmul(
                                ps2[:, :nr, :], lhsT=w2_sb[:, idx, :], rhs=rhs,
                                start=(idx == 0), stop=(idx == 8))
                        nc.scalar.activation(ker_st[0:18, g0 - 2:g0 - 2 + nr, :],
                                             ps2[:, :nr], Act.Identity,
                                             bias=b2_sb, scale=1.0)
                        if p > 0 and ci % 2 == 1:
                            drain_acc(1)

                    # DMA xbar transpose to pixel partitions, then repack
                    # on gpsimd to kt2[q, ti, r] (r innermost for the
                    # product broadcast)
                    if p < 2:
                        ktA = ktap.tile([W, SLAB, 32], f16, tag="ktA")
                        nc.sync.dma_start_transpose(
                            out=ktA, in_=ker_st.rearrange("ti r q -> ti (r q)"))
                        nc.gpsimd.tensor_copy(
                            kt2[p], ktA.rearrange("q r ti -> q ti r"))
                    else:
                        ktA = ktap.tile([W, 16, 32], f16, tag="ktA2")
                        nc.sync.dma_start_transpose(
                            out=ktA,
                            in_=ker_st[:, 0:16, :].rearrange("ti r q -> ti (r q)"))
                        # halves: taps 0-8 = kernel rows 0-15, taps 9-17 =
                        # kernel rows 16-31
                        nc.gpsimd.tensor_copy(
                            kt2[2][:, 0:9, 0:16],
                            ktA[:, :, 0:9].rearrange("q r ti -> q ti r"))
                        nc.gpsimd.tensor_copy(
                            kt2[2][:, 0:9, 16:32],
                            ktA[:, :, 9:18].rearrange("q r ti -> q ti r"))

                    if p == 0:
                        emit_u_part2()

                    # per-pass kernel sum for the normalization coefficient
                    # (must read the PRE-merge kernel values)
                    nt = 18 if p < 2 else 9
                    nc.vector.tensor_reduce(
                        r_p[p], kt2[p].rearrange("q ti r -> q r ti")[:, :, 0:nt],
                        axis=Ax.X, op=Alu.add)

                    # fold the edge-replicated dj=0 (q=0) / dj=2 (q=127)
                    # terms into the dj=1 kernel: they multiply the same x
                    # element as the dj=1 term at that column.  Engines
                    # can't start mid-partition, so mask with a per-
                    # partition one-hot: dj1 += onehot(edge) * dj_edge
                    ev = kt2[p][:, 0:nt, :].rearrange("q (a b) r -> q a b r",
                                                      b=3)
                    nc.vector.scalar_tensor_tensor(
                        ev[:, :, 1, :], ev[:, :, 0, :], em_sb[:, 0:1],
                        ev[:, :, 1, :], Alu.mult, Alu.add)
                    nc.vector.scalar_tensor_tensor(
                        ev[:, :, 1, :], ev[:, :, 2, :], em_sb[:, 1:2],
                        ev[:, :, 1, :], Alu.mult, Alu.add)

                    # partition-shifted kernel copies for dj=0 / dj=2
                    # (products run at the source pixel; the PE accumulate
                    # shifts them into place with eye(k=+-1))
                    ktp_t = kshp.tile([W, 32, SLAB], f16, tag="kp")
                    ktm_t = kshp.tile([W, 32, SLAB], f16, tag="km")
                    nc.gpsimd.dma_start(out=ktp_t[0:127], in_=kt2[p][1:128])
                    nc.gpsimd.dma_start(out=ktp_t[127:128], in_=kt2[p][127:128])
                    nc.gpsimd.dma_start(out=ktm_t[1:128], in_=kt2[p][0:127])
                    nc.gpsimd.dma_start(out=ktm_t[0:1], in_=kt2[p][0:1])

                    # dynamic-filter products: pairs (same dj) -> one DVE
                    # add -> PE accumulate (drained interleaved with conv)
                    if p < 2:
                        groups = [[(2 * p + fi, fi * 9 + 3 * di + dj, di, dj)
                                   for fi in (0, 1)]
                                  for dj in (1, 0, 2) for di in range(3)]
                    else:
                        groups = []
                        for dj in (1, 0, 2):
                            terms = [(4, 3 * di + dj, di, dj)
                                     for di in range(3)]
                            groups += [terms[0:2], terms[2:3]]
                    for g in groups:
                        prods = []
                        for (f, ti, di, dj) in g:
                            src = (ktp_t, kt2[p], ktm_t)[dj]
                            kb = src[:, ti, :].unsqueeze(1)\
                                .broadcast_to((W, DIM, SLAB))
                            if di == 1:
                                xs = xts1[:, f, :, :]
                            else:
                                xs = xt1[:, f, :, di:di + SLAB]
                            prod = tp.tile([W, DIM, SLAB], f16, tag="ts")
                            nc.vector.tensor_tensor(prod, xs, kb, Alu.mult)
                            prods.append(prod)
                        if len(prods) == 2:
                            nc.vector.tensor_tensor(prods[0], prods[0],
                                                    prods[1], Alu.add)
                        pending.append((prods[0], g[0][3]))

                # --- normalization: c = 1/45 - mean(ker); out += c * S ---
                nc.vector.tensor_tensor(r_p[0], r_p[0], r_p[1], Alu.add)
                nc.vector.tensor_tensor(r_p[0], r_p[0], r_p[2], Alu.add)
                c_sb = ktp.tile([W, SLAB], f32, name="c_sb")
                nc.vector.tensor_scalar(c_sb, r_p[0], -1.0 / 45.0, 1.0 / 45.0,
                                        Alu.mult, Alu.add)
                c_bf = ktp.tile([W, SLAB], f16, name="c_bf")
                nc.vector.tensor_copy(c_bf, c_sb)
                cs_prod = tp.tile([W, DIM, SLAB], f16, tag="ts")
                cb = c_bf.unsqueeze(1).broadcast_to((W, DIM, SLAB))
                nc.vector.tensor_tensor(cs_prod, sv, cb, Alu.mult)
                pending.append((cs_prod, 1))

                final_drain()

                # drain acc -> fp16, then DMA-xbar transposes to DRAM
                acc_sb = ktp.tile([W, DIM * SLAB], f16, name="acc_sb")
                for cc in range(4):
                    sl = slice(512 * cc, 512 * (cc + 1))
                    nc.scalar.activation(acc_sb[:, sl], acc[:, sl],
                                         Act.Copy, scale=1.0)
                out_rc = out_d.rearrange("c r w -> (c r) w")
                for k in range(16):
                    so = slice(128 * k, 128 * (k + 1))
                    ob = obp.tile([128, 128], f16, tag="ob")
                    eng = nc.sync if k % 2 == 0 else nc.scalar
                    eng.dma_start_transpose(out=ob, in_=acc_sb[:, so])
                    eng.dma_start(out=out_rc[so], in_=ob)

    return nc


def _get_program():
    if "nc" not in _PROGRAM_CACHE:
        nc = _build_program()
        nc.finalize()
        _PROGRAM_CACHE["nc"] = nc
    return _PROGRAM_CACHE["nc"]


def _host_prep(x, w1, b1, w2, b2):
    """Build the 8 per-core input maps from full inputs."""
    x = np.asarray(x, dtype=np.float32)
    w1 = np.asarray(w1, dtype=np.float32)
    b1 = np.asarray(b1, dtype=np.float32)
    w2 = np.asarray(w2, dtype=np.float32)
    b2 = np.asarray(b2, dtype=np.float32)
    f16 = np.float16

    # block-diagonal packed weights: passes 0/1 = 2 frames, pass 2 = the
    # two spatial halves of frame 4 -> identical weight matrices
    w1t = w1.transpose(1, 2, 3, 0).reshape(DIM, 9, DIM)   # [ci, tap, o]
    w2t = w2.transpose(1, 2, 3, 0).reshape(DIM, 9, 9)
    w1a = np.zeros((128, 9, 128), np.float32)
    w1a[0:64, :, 0:64] = w1t
    w1a[64:128, :, 64:128] = w1t
    w2a = np.zeros((128, 9, 18), np.float32)
    w2a[0:64, :, 0:9] = w2t
    w2a[64:128, :, 9:18] = w2t

    b1r = np.concatenate([b1, b1]).reshape(128, 1).astype(np.float32)
    b2r = np.concatenate([b2, b2]).reshape(18, 1).astype(np.float32)
    idf = np.eye(128, dtype=f16)
    w1a = w1a.astype(f16)
    w2a = w2a.astype(f16)

    in_maps = []
    for core in range(NCORES):
        b, s = divmod(core, 4)
        r0 = s * SLAB
        # passes 0/1 conv input: frames (2p, 2p+1) on the partition
        # halves, x rows r0-2 .. r0+33 zero padded, cols -1..128 zero
        xc01 = np.zeros((2, 128, GH, GW), np.float32)
        lo = max(0, r0 - 2)
        hi = min(H, r0 + 34)
        for p in range(2):
            for f in range(2):
                t = 2 * p + f
                xc01[p, f * 64:(f + 1) * 64,
                     lo - (r0 - 2):hi - (r0 - 2), 1:129] = x[b, :, t, lo:hi, :]
        # pass 2: frame 4 split into two 16-row halves on the partition
        # halves (plus conv halo)
        xc2 = np.zeros((128, GH2, GW), np.float32)
        for h2 in range(2):
            bx = r0 - 2 if h2 == 0 else r0 + 14
            lo2 = max(0, bx)
            hi2 = min(H, bx + GH2)
            xc2[h2 * 64:(h2 + 1) * 64, lo2 - bx:hi2 - bx, 1:129] = \
                x[b, :, 4, lo2:hi2, :]
        # filter input, pixel-partition, center (dj=1) copy only; xts =
        # the r0-based row window so di=1 product slices start 4B-aligned
        rows = np.clip(np.arange(r0 - 1, r0 + 33), 0, H - 1)
        xt = x[b][:, :, rows, :].transpose(3, 1, 0, 2)          # (w,t,c,34)
        xts = x[b][:, :, r0:r0 + 32, :].transpose(3, 1, 0, 2)   # (w,t,c,32)
        # shifted identities for the dj=0/dj=2 accumulates
        sm = np.zeros((128, 2, 128), np.float32)
        sm[0:127, 0, :] = np.eye(128, dtype=np.float32)[1:128]   # m = p+1
        sm[1:128, 1, :] = np.eye(128, dtype=np.float32)[0:127]   # m = p-1
        em = np.zeros((128, 2), np.float32)
        em[0, 0] = 1.0      # q=0 edge (dj=0 term folds into dj=1)
        em[127, 1] = 1.0    # q=127 edge (dj=2 term folds into dj=1)
        # conv2 zero-pad masks for y rows outside the image
        ym = np.ones((128, 2), np.float32)
        if s == 0:
            ym[:, 0] = 0.0
        if s == 3:
            ym[:, 1] = 0.0
        ym2 = np.ones((128, 2), np.float32)
        if s == 0:
            ym2[0:64, 0] = 0.0
        if s == 3:
            ym2[64:128, 1] = 0.0
        in_maps.append({
            "xc01": xc01.astype(f16), "xc2": xc2.astype(f16),
            "xt": xt.astype(f16), "xts": xts.astype(f16),
            "w1": w1a, "w2": w2a, "b1r": b1r, "b2r": b2r,
            "ym": ym, "ym2": ym2, "idf": idf, "sm": sm.astype(f16),
            "em": em,
        })
    return in_maps


def kernel(x, w1, b1, w2, b2):
    from concourse.bass_utils import run_bass_kernel_spmd

    nc = _get_program()
    in_maps = _host_prep(x, w1, b1, w2, b2)
    res = run_bass_kernel_spmd(nc, in_maps, list(range(NCORES)))
    out = np.zeros((2, DIM, H, W), dtype=np.float32)
    for core in range(NCORES):
        b, s = divmod(core, 4)
        out[b, :, s * SLAB:(s + 1) * SLAB, :] = \
            res.results[core]["out"].astype(np.float32)
    return out


# revision 25
# speedup vs baseline: 1.0915x; 1.0589x over previous
"""Trainium2 Bass kernel for nn_DynamicFiltering (v2).

Computation (per batch b):
  y  = LeakyReLU(conv2d(x_t, w1, b1), 0.2)        per frame t
  ker = conv2d(y, w2, b2)                          (t, 9, h, w)
  ker = ker - mean_K(ker) + 1/45                   per-pixel over K = 45
  out[c,h,w] = sum_{t,k1,k2} x_edge[c,t,h+k1-1,w+k2-1] * ker[t,k1,k2][h,w]

Sharding: 8 cores = 2 batches x 4 H-slabs of 32 rows.

Structure (v2, vs the 234us bf16 baseline):
  - all 16-bit data is fp16 (same speed as bf16, ~8x less quant error)
  - conv1 leaky relu is a single Act Prelu(alpha=0.2) drain (verified on
    HW: Prelu honors alpha, Lrelu hardcodes 0.01) -> GpSimd fully freed
  - pass 2 (lone frame 4) is spatially halved: both partition halves
    carry frame-4 channels over half the rows, same block-diag weights
    as the 2-frame passes -> conv1/conv2 pass-2 matmul cycles halve
  - dynamic-filter products run on DVE in 2x fp16 mode: the di=1 row
    window is served by a separate host copy (xts) so every slice start
    is 4B-aligned
  - only the CENTER (dj=1) pixel-partition x copy is loaded; the dj
    column shift moves to the kernel side: kt2 is partition-shifted by
    +-1 via SBUF-SBUF DMA (2KB/partition vs 40KB for x copies) and the
    PE accumulate uses shifted identity matrices (eye(k=+-1)).  The
    edge-replication terms (q=0 dj=0, q=127 dj=2) multiply the same x
    element as the dj=1 term, so they fold into the dj=1 kernel's edge
    values with one tiny DVE add per edge per pass
  - no tree reduction / pass sums: product pairs (same dj) are added
    once on DVE, then PE matmuls accumulate each pair tile into a
    persistent 4-bank PSUM accumulator, interleaved with the next
    pass's conv matmuls (PSUM: 2 conv1 + 2 conv2 + 4 acc banks = 8)
  - normalization term c*S as in v1 (U chain: sums on DVE in the
    pre-product idle window, shifts/box-sums on GpSimd)
  - output: acc -> fp16 -> 16 DMA-xbar transposes -> DRAM (host casts
    to fp32); no PE transposes, no fp32 identity
  - startup: w1 + pass-0 conv input bands issued first on the sync
    HWDGE ring; all other constants + pass-1/2 inputs on the act ring
"""

import numpy as np

DIM = 64
T = 5
H = 128
W = 128
SLAB = 32          # output rows per core
NCORES = 8
GH = 36            # conv grid rows, passes 0/1: slab + 2*2 halo
GH2 = 20           # conv grid rows, pass 2 halves: 16 + 2*2 halo
GW = 130           # conv grid cols: W + 2
FR = 34            # filter rows: slab + 2 halo
NPASS = 3

_PROGRAM_CACHE = {}

C1_CHUNKS = [(1 + 4 * i, 4) for i in range(8)] + [(33, 2)]
C2_CHUNKS = [(2 + 4 * i, 4) for i in range(8)]
C1_CHUNKS2 = [(1 + 4 * i, 4) for i in range(4)] + [(17, 2)]
C2_CHUNKS2 = [(2 + 4 * i, 4) for i in range(4)]


def _build_program():
    import concourse.bacc as bacc
    import concourse.mybir as mybir
    from concourse.tile import TileContext

    f32 = mybir.dt.float32
    f16 = mybir.dt.float16
    u16 = mybir.dt.uint16
    Act = mybir.ActivationFunctionType
    Alu = mybir.AluOpType
    Ax = mybir.AxisListType

    nc = bacc.Bacc("TRN2", debug=False)

    xc01_d = nc.dram_tensor("xc01", [2, 128, GH, GW], f16, kind="ExternalInput").ap()
    xc2_d = nc.dram_tensor("xc2", [128, GH2, GW], f16, kind="ExternalInput").ap()
    xt_d = nc.dram_tensor("xt", [W, T, DIM, FR], f16, kind="ExternalInput").ap()
    xts_d = nc.dram_tensor("xts", [W, T, DIM, SLAB], f16, kind="ExternalInput").ap()
    sm_d = nc.dram_tensor("sm", [128, 2, 128], f16, kind="ExternalInput").ap()
    em_d = nc.dram_tensor("em", [128, 2], f32, kind="ExternalInput").ap()
    w1_d = nc.dram_tensor("w1", [128, 9, 128], f16, kind="ExternalInput").ap()
    w2_d = nc.dram_tensor("w2", [128, 9, 18], f16, kind="ExternalInput").ap()
    b1_d = nc.dram_tensor("b1r", [128, 1], f32, kind="ExternalInput").ap()
    b2_d = nc.dram_tensor("b2r", [18, 1], f32, kind="ExternalInput").ap()
    ym_d = nc.dram_tensor("ym", [128, 2], f32, kind="ExternalInput").ap()
    ym2_d = nc.dram_tensor("ym2", [128, 2], f32, kind="ExternalInput").ap()
    idf_d = nc.dram_tensor("idf", [128, 128], f16, kind="ExternalInput").ap()
    # out[b_, a, q] = acc[q, 128*a + b_]; the host unscrambles (c r) =
    # 128*a + b_ back to [c, r] (one contiguous DMA instead of 16)
    out_d = nc.dram_tensor("out", [128, 16, W], f16, kind="ExternalOutput").ap()

    with TileContext(nc) as tc:
        with (
            tc.tile_pool(name="consts", bufs=1) as cpool,
            tc.tile_pool(name="xtp", bufs=1) as xtp,
            tc.tile_pool(name="xcp", bufs=2) as xcp,
            tc.tile_pool(name="xc2p", bufs=1) as xc2p,
            tc.tile_pool(name="yp", bufs=3) as yp,
            tc.tile_pool(name="ksh", bufs=2) as kshp,
            tc.tile_pool(name="kst", bufs=1) as kstp,
            tc.tile_pool(name="kta", bufs=1) as ktap,
            tc.tile_pool(name="ktp", bufs=1) as ktp,
            tc.tile_pool(name="up", bufs=1) as up,
            tc.tile_pool(name="tp", bufs=8) as tp,
            tc.tile_pool(name="obp", bufs=1) as obp,
        ):
            # ---- startup DMAs: sync ring carries only what gates the ----
            # ---- first conv1 matmuls (w1 + pass-0 input bands)        ----
            w1_sb = cpool.tile([128, 9, 128], f16)
            nc.sync.dma_start(out=w1_sb, in_=w1_d)

            def load_xc01(p, eng):
                t = xcp.tile([128, GH, GW], f16, tag="xc")
                for r0b, r1b in ((0, 8), (8, 16), (16, 24), (24, 32), (32, 36)):
                    eng.dma_start(out=t[:, r0b:r1b], in_=xc01_d[p, :, r0b:r1b])
                return t

            xc_p0 = load_xc01(0, nc.sync)

            # filter inputs on the sync HWDGE ring AFTER the pass-0 conv
            # bands: per-ring FIFO means the bands drain at full
            # bandwidth first (on the SWDGE ring their huge descriptors
            # monopolized the SDMA engines and stalled conv1 pass 0)
            xt1 = xtp.tile([W, T, DIM, FR], f16, name="xt1")
            xts1 = xtp.tile([W, T, DIM, SLAB], f16, name="xts1")
            nc.sync.dma_start(out=xt1, in_=xt_d)
            nc.sync.dma_start(out=xts1, in_=xts_d)

            # act HWDGE ring: everything else, in need-order
            b1_sb = cpool.tile([128, 1], f32)
            nc.scalar.dma_start(out=b1_sb, in_=b1_d)
            w2_sb = cpool.tile([128, 9, 18], f16)
            nc.scalar.dma_start(out=w2_sb, in_=w2_d)
            b2_sb = cpool.tile([18, 1], f32)
            nc.scalar.dma_start(out=b2_sb, in_=b2_d)
            ym_sb = cpool.tile([128, 2], f32)
            nc.scalar.dma_start(out=ym_sb, in_=ym_d)
            ym2_sb = cpool.tile([128, 2], f32)
            nc.scalar.dma_start(out=ym2_sb, in_=ym2_d)
            idf_sb = cpool.tile([128, 128], f16)
            nc.scalar.dma_start(out=idf_sb, in_=idf_d)
            sm_sb = cpool.tile([128, 2, 128], f16)
            nc.scalar.dma_start(out=sm_sb, in_=sm_d)
            em_sb = cpool.tile([128, 2], f32)
            nc.scalar.dma_start(out=em_sb, in_=em_d)
            xc_p1 = load_xc01(1, nc.scalar)
            xc_p2 = xc2p.tile([128, GH2, GW], f16)
            for r0b, r1b in ((0, 8), (8, 16), (16, 20)):
                nc.scalar.dma_start(out=xc_p2[:, r0b:r1b], in_=xc2_d[:, r0b:r1b])

            # conv2 -> kernel staging (ti on partitions)
            ker_st = kstp.tile([32, SLAB, W], f16)
            nc.gpsimd.memset(ker_st.bitcast(u16), 0)

            # y tiles pre-allocated; edge cols zeroed up front on gpsimd
            y_t = [yp.tile([128, GH, GW], f16, name=f"y{p}", tag="y")
                   for p in range(3)]
            for p in range(3):
                nr = 34 if p < 2 else 18
                nc.gpsimd.memset(y_t[p][:, 1:1 + nr, 0:1].bitcast(u16), 0)
                nc.gpsimd.memset(y_t[p][:, 1:1 + nr, 129:130].bitcast(u16), 0)

            kt2 = [ktp.tile([W, 32, SLAB], f16, name=f"kt2_{p}")
                   for p in range(NPASS)]

            # U chain part 1 on DVE: fills the idle window before the
            # first products (xt arrives ~35us, first kernels ~41us)
            u_c = up.tile([W, DIM, FR], f16, name="u_c")
            u_m1 = up.tile([W, DIM, FR], f16, name="u_m1")
            u_p1 = up.tile([W, DIM, FR], f16, name="u_p1")
            sv = up.tile([W, DIM, SLAB], f16, name="sv")
            nc.vector.tensor_tensor(u_c, xt1[:, 0], xt1[:, 1], Alu.add)
            for t_i in (2, 3, 4):
                nc.vector.tensor_tensor(u_c, u_c, xt1[:, t_i], Alu.add)

            def emit_u_part2():
                # partition-shifted copies via DMA (engines are lockstep;
                # only DMA can shift partitions); edges replicate
                nc.gpsimd.dma_start(out=u_m1[1:128], in_=u_c[0:127])
                nc.gpsimd.dma_start(out=u_m1[0:1], in_=u_c[0:1])
                nc.gpsimd.dma_start(out=u_p1[0:127], in_=u_c[1:128])
                nc.gpsimd.dma_start(out=u_p1[127:128], in_=u_c[127:128])
                nc.gpsimd.tensor_tensor(u_m1, u_c, u_m1, Alu.add)
                nc.gpsimd.tensor_tensor(u_m1, u_m1, u_p1, Alu.add)
                nc.gpsimd.tensor_tensor(sv, u_m1[:, :, 0:SLAB],
                                        u_m1[:, :, 1:SLAB + 1], Alu.add)
                nc.gpsimd.tensor_tensor(sv, sv, u_m1[:, :, 2:SLAB + 2], Alu.add)

            r_p = [ktp.tile([W, SLAB], f32, name=f"r{p}") for p in range(NPASS)]

            with (
                tc.tile_pool(name="ps1", bufs=2, space="PSUM") as ps1p,
                tc.tile_pool(name="ps2", bufs=2, space="PSUM") as ps2p,
                tc.tile_pool(name="acc", bufs=1, space="PSUM") as accp,
            ):
                acc = accp.tile([W, DIM * SLAB], f32)
                pending = []          # (tile, dj) awaiting PE accumulate
                acc_state = {"first": True}

                def acc_mm(tile, dj, last):
                    # dj=1: plain identity; dj=0/2: shifted identity
                    # applies the +-1 pixel-column shift of the patches
                    lhs = (sm_sb[:, 0, :], idf_sb, sm_sb[:, 1, :])[dj]
                    fl = tile.rearrange("q c r -> q (c r)")
                    for cc in range(4):
                        sl = slice(512 * cc, 512 * (cc + 1))
                        nc.tensor.matmul(acc[:, sl], lhsT=lhs, rhs=fl[:, sl],
                                         start=acc_state["first"], stop=last)
                    acc_state["first"] = False

                def drain_acc(n):
                    for _ in range(min(n, len(pending))):
                        tile, dj = pending.pop(0)
                        acc_mm(tile, dj, False)

                def final_drain():
                    while len(pending) > 1:
                        drain_acc(1)
                    tile, dj = pending.pop(0)
                    acc_mm(tile, dj, True)

                for p in range(NPASS):
                    xc_f = (xc_p0, xc_p1, xc_p2)[p]
                    y_f = y_t[p]
                    c1 = C1_CHUNKS if p < 2 else C1_CHUNKS2
                    c2 = C2_CHUNKS if p < 2 else C2_CHUNKS2

                    # conv1 + leaky relu (single Prelu drain per chunk)
                    for ci, (g0, nr) in enumerate(c1):
                        ps = ps1p.tile([128, 4, W], f32, tag="ps1")
                        for idx in range(9):
                            di, dj = divmod(idx, 3)
                            rhs = xc_f[:, g0 + di - 1:g0 + di - 1 + nr,
                                       dj:dj + W]
                            nc.tensor.matmul(
                                ps[:, :nr, :], lhsT=w1_sb[:, idx, :], rhs=rhs,
                                start=(idx == 0), stop=(idx == 8))
                        nc.scalar.activation(y_f[:, g0:g0 + nr, 1:129],
                                             ps[:, :nr], Act.Prelu,
                                             bias=b1_sb, scale=1.0, alpha=0.2)
                        # interleave pending accumulates from the previous
                        # pass (PE slightly ahead of DVE -> no stall)
                        if p > 0 and ci >= 2 and ci % 2 == 0:
                            drain_acc(1)

                    # conv2 zero-pads rows outside the image: kill y halo
                    ymm = ym_sb if p < 2 else ym2_sb
                    hrow = 34 if p < 2 else 18
                    nc.scalar.activation(y_f[:, 1:2, 1:129],
                                         y_f[:, 1:2, 1:129],
                                         Act.Copy, scale=ymm[:, 0:1])
                    nc.scalar.activation(y_f[:, hrow:hrow + 1, 1:129],
                                         y_f[:, hrow:hrow + 1, 1:129],
                                         Act.Copy, scale=ymm[:, 1:2])

                    # conv2 -> ker_st[ti, r, q]
                    for ci, (g0, nr) in enumerate(c2):
                        ps2 = ps2p.tile([18, 4, W], f32, tag="ps2")
                        for idx in range(9):
                            di, dj = divmod(idx, 3)
                            rhs = y_f[:, g0 + di - 1:g0 + di - 1 + nr,
                                      dj:dj + W]
                            nc.tensor.matmul(
                                ps2[:, :nr, :], lhsT=w2_sb[:, idx, :], rhs=rhs,
                                start=(idx == 0), stop=(idx == 8))
                        nc.scalar.activation(ker_st[0:18, g0 - 2:g0 - 2 + nr, :],
                                             ps2[:, :nr], Act.Identity,
                                             bias=b2_sb, scale=1.0)
                        if p > 0 and ci % 2 == 1:
                            drain_acc(1)

                    # DMA xbar transpose to pixel partitions, then repack
                    # on gpsimd to kt2[q, ti, r] (r innermost for the
                    # product broadcast)
                    if p < 2:
                        ktA = ktap.tile([W, SLAB, 32], f16, tag="ktA")
                        nc.sync.dma_start_transpose(
                            out=ktA, in_=ker_st.rearrange("ti r q -> ti (r q)"))
                        nc.gpsimd.tensor_copy(
                            kt2[p], ktA.rearrange("q r ti -> q ti r"))
                    else:
                        ktA = ktap.tile([W, 16, 32], f16, tag="ktA2")
                        nc.sync.dma_start_transpose(
                            out=ktA,
                            in_=ker_st[:, 0:16, :].rearrange("ti r q -> ti (r q)"))
                        # halves: taps 0-8 = kernel rows 0-15, taps 9-17 =
                        # kernel rows 16-31
                        nc.gpsimd.tensor_copy(
                            kt2[2][:, 0:9, 0:16],
                            ktA[:, :, 0:9].rearrange("q r ti -> q ti r"))
                        nc.gpsimd.tensor_copy(
                            kt2[2][:, 0:9, 16:32],
                            ktA[:, :, 9:18].rearrange("q r ti -> q ti r"))

                    if p == 0:
                        emit_u_part2()

                    # per-pass kernel sum for the normalization coefficient
                    # (must read the PRE-merge kernel values)
                    nt = 18 if p < 2 else 9
                    nc.vector.tensor_reduce(
                        r_p[p], kt2[p].rearrange("q ti r -> q r ti")[:, :, 0:nt],
                        axis=Ax.X, op=Alu.add)

                    # fold the edge-replicated dj=0 (q=0) / dj=2 (q=127)
                    # terms into the dj=1 kernel: they multiply the same x
                    # element as the dj=1 term at that column.  Engines
                    # can't start mid-partition, so mask with a per-
                    # partition one-hot: dj1 += onehot(edge) * dj_edge
                    ev = kt2[p][:, 0:nt, :].rearrange("q (a b) r -> q a b r",
                                                      b=3)
                    nc.vector.scalar_tensor_tensor(
                        ev[:, :, 1, :], ev[:, :, 0, :], em_sb[:, 0:1],
                        ev[:, :, 1, :], Alu.mult, Alu.add)
                    nc.vector.scalar_tensor_tensor(
                        ev[:, :, 1, :], ev[:, :, 2, :], em_sb[:, 1:2],
                        ev[:, :, 1, :], Alu.mult, Alu.add)

                    # partition-shifted kernel copies for dj=0 / dj=2
                    # (products run at the source pixel; the PE accumulate
                    # shifts them into place with eye(k=+-1))
                    ktp_t = kshp.tile([W, 32, SLAB], f16, tag="kp")
                    ktm_t = kshp.tile([W, 32, SLAB], f16, tag="km")
                    nc.gpsimd.dma_start(out=ktp_t[0:127], in_=kt2[p][1:128])
                    nc.gpsimd.dma_start(out=ktp_t[127:128], in_=kt2[p][127:128])
                    nc.gpsimd.dma_start(out=ktm_t[1:128], in_=kt2[p][0:127])
                    nc.gpsimd.dma_start(out=ktm_t[0:1], in_=kt2[p][0:1])

                    # dynamic-filter products: pairs (same dj) -> one DVE
                    # add -> PE accumulate (drained interleaved with conv)
                    if p < 2:
                        groups = [[(2 * p + fi, fi * 9 + 3 * di + dj, di, dj)
                                   for fi in (0, 1)]
                                  for dj in (1, 0, 2) for di in range(3)]
                    else:
                        groups = []
                        for dj in (1, 0, 2):
                            terms = [(4, 3 * di + dj, di, dj)
                                     for di in range(3)]
                            groups += [terms[0:2], terms[2:3]]
                    for g in groups:
                        prods = []
                        for (f, ti, di, dj) in g:
                            src = (ktp_t, kt2[p], ktm_t)[dj]
                            kb = src[:, ti, :].unsqueeze(1)\
                                .broadcast_to((W, DIM, SLAB))
                            if di == 1:
                                xs = xts1[:, f, :, :]
                            else:
                                xs = xt1[:, f, :, di:di + SLAB]
                            prod = tp.tile([W, DIM, SLAB], f16, tag="ts")
                            nc.vector.tensor_tensor(prod, xs, kb, Alu.mult)
                            prods.append(prod)
                        if len(prods) == 2:
                            nc.vector.tensor_tensor(prods[0], prods[0],
                                                    prods[1], Alu.add)
                        pending.append((prods[0], g[0][3]))

                # --- normalization: c = 1/45 - mean(ker); out += c * S ---
                nc.vector.tensor_tensor(r_p[0], r_p[0], r_p[1], Alu.add)
                nc.vector.tensor_tensor(r_p[0], r_p[0], r_p[2], Alu.add)
                c_sb = ktp.tile([W, SLAB], f32, name="c_sb")
                nc.vector.tensor_scalar(c_sb, r_p[0], -1.0 / 45.0, 1.0 / 45.0,
                                        Alu.mult, Alu.add)
                c_bf = ktp.tile([W, SLAB], f16, name="c_bf")
                nc.vector.tensor_copy(c_bf, c_sb)
                cs_prod = tp.tile([W, DIM, SLAB], f16, tag="ts")
                cb = c_bf.unsqueeze(1).broadcast_to((W, DIM, SLAB))
                nc.vector.tensor_tensor(cs_prod, sv, cb, Alu.mult)
                pending.append((cs_prod, 1))

                final_drain()

                # drain acc -> fp16, then DMA-xbar transposes to DRAM
                acc_sb = ktp.tile([W, DIM * SLAB], f16, name="acc_sb")
                for cc in range(4):
                    sl = slice(512 * cc, 512 * (cc + 1))
                    nc.scalar.activation(acc_sb[:, sl], acc[:, sl],
                                         Act.Copy, scale=1.0)
                ob = obp.tile([128, 16, W], f16, tag="ob")
                nc.sync.dma_start_transpose(out=ob, in_=acc_sb)
                nc.sync.dma_start(out=out_d, in_=ob)

    return nc


def _get_program():
    if "nc" not in _PROGRAM_CACHE:
        nc = _build_program()
        nc.finalize()
        _PROGRAM_CACHE["nc"] = nc
    return _PROGRAM_CACHE["nc"]


def _host_prep(x, w1, b1, w2, b2):
    """Build the 8 per-core input maps from full inputs."""
    x = np.asarray(x, dtype=np.float32)
    w1 = np.asarray(w1, dtype=np.float32)
    b1 = np.asarray(b1, dtype=np.float32)
    w2 = np.asarray(w2, dtype=np.float32)
    b2 = np.asarray(b2, dtype=np.float32)
    f16 = np.float16

    # block-diagonal packed weights: passes 0/1 = 2 frames, pass 2 = the
    # two spatial halves of frame 4 -> identical weight matrices
    w1t = w1.transpose(1, 2, 3, 0).reshape(DIM, 9, DIM)   # [ci, tap, o]
    w2t = w2.transpose(1, 2, 3, 0).reshape(DIM, 9, 9)
    w1a = np.zeros((128, 9, 128), np.float32)
    w1a[0:64, :, 0:64] = w1t
    w1a[64:128, :, 64:128] = w1t
    w2a = np.zeros((128, 9, 18), np.float32)
    w2a[0:64, :, 0:9] = w2t
    w2a[64:128, :, 9:18] = w2t

    b1r = np.concatenate([b1, b1]).reshape(128, 1).astype(np.float32)
    b2r = np.concatenate([b2, b2]).reshape(18, 1).astype(np.float32)
    idf = np.eye(128, dtype=f16)
    w1a = w1a.astype(f16)
    w2a = w2a.astype(f16)

    in_maps = []
    for core in range(NCORES):
        b, s = divmod(core, 4)
        r0 = s * SLAB
        # passes 0/1 conv input: frames (2p, 2p+1) on the partition
        # halves, x rows r0-2 .. r0+33 zero padded, cols -1..128 zero
        xc01 = np.zeros((2, 128, GH, GW), np.float32)
        lo = max(0, r0 - 2)
        hi = min(H, r0 + 34)
        for p in range(2):
            for f in range(2):
                t = 2 * p + f
                xc01[p, f * 64:(f + 1) * 64,
                     lo - (r0 - 2):hi - (r0 - 2), 1:129] = x[b, :, t, lo:hi, :]
        # pass 2: frame 4 split into two 16-row halves on the partition
        # halves (plus conv halo)
        xc2 = np.zeros((128, GH2, GW), np.float32)
        for h2 in range(2):
            bx = r0 - 2 if h2 == 0 else r0 + 14
            lo2 = max(0, bx)
            hi2 = min(H, bx + GH2)
            xc2[h2 * 64:(h2 + 1) * 64, lo2 - bx:hi2 - bx, 1:129] = \
                x[b, :, 4, lo2:hi2, :]
        # filter input, pixel-partition, center (dj=1) copy only; xts =
        # the r0-based row window so di=1 product slices start 4B-aligned
        rows = np.clip(np.arange(r0 - 1, r0 + 33), 0, H - 1)
        xt = x[b][:, :, rows, :].transpose(3, 1, 0, 2)          # (w,t,c,34)
        xts = x[b][:, :, r0:r0 + 32, :].transpose(3, 1, 0, 2)   # (w,t,c,32)
        # shifted identities for the dj=0/dj=2 accumulates
        sm = np.zeros((128, 2, 128), np.float32)
        sm[0:127, 0, :] = np.eye(128, dtype=np.float32)[1:128]   # m = p+1
        sm[1:128, 1, :] = np.eye(128, dtype=np.float32)[0:127]   # m = p-1
        em = np.zeros((128, 2), np.float32)
        em[0, 0] = 1.0      # q=0 edge (dj=0 term folds into dj=1)
        em[127, 1] = 1.0    # q=127 edge (dj=2 term folds into dj=1)
        # conv2 zero-pad masks for y rows outside the image
        ym = np.ones((128, 2), np.float32)
        if s == 0:
            ym[:, 0] = 0.0
        if s == 3:
            ym[:, 1] = 0.0
        ym2 = np.ones((128, 2), np.float32)
        if s == 0:
            ym2[0:64, 0] = 0.0
        if s == 3:
            ym2[64:128, 1] = 0.0
        in_maps.append({
            "xc01": xc01.astype(f16), "xc2": xc2.astype(f16),
            "xt": xt.astype(f16), "xts": xts.astype(f16),
            "w1": w1a, "w2": w2a, "b1r": b1r, "b2r": b2r,
            "ym": ym, "ym2": ym2, "idf": idf, "sm": sm.astype(f16),
            "em": em,
        })
    return in_maps


def kernel(x, w1, b1, w2, b2):
    from concourse.bass_utils import run_bass_kernel_spmd

    nc = _get_program()
    in_maps = _host_prep(x, w1, b1, w2, b2)
    res = run_bass_kernel_spmd(nc, in_maps, list(range(NCORES)))
    out = np.zeros((2, DIM, H, W), dtype=np.float32)
    for core in range(NCORES):
        b, s = divmod(core, 4)
        # device layout: o[b_, a, q] = result[(c r) = 128*a + b_, q]
        o = res.results[core]["out"].astype(np.float32)
        o = o.transpose(1, 0, 2).reshape(DIM, SLAB, W)
        out[b, :, s * SLAB:(s + 1) * SLAB, :] = o
    return out


# revision 26
# speedup vs baseline: 1.0928x; 1.0012x over previous
"""Trainium2 Bass kernel for nn_DynamicFiltering (v2).

Computation (per batch b):
  y  = LeakyReLU(conv2d(x_t, w1, b1), 0.2)        per frame t
  ker = conv2d(y, w2, b2)                          (t, 9, h, w)
  ker = ker - mean_K(ker) + 1/45                   per-pixel over K = 45
  out[c,h,w] = sum_{t,k1,k2} x_edge[c,t,h+k1-1,w+k2-1] * ker[t,k1,k2][h,w]

Sharding: 8 cores = 2 batches x 4 H-slabs of 32 rows.

Structure (v2, vs the 234us bf16 baseline):
  - all 16-bit data is fp16 (same speed as bf16, ~8x less quant error)
  - conv1 leaky relu is a single Act Prelu(alpha=0.2) drain (verified on
    HW: Prelu honors alpha, Lrelu hardcodes 0.01) -> GpSimd fully freed
  - pass 2 (lone frame 4) is spatially halved: both partition halves
    carry frame-4 channels over half the rows, same block-diag weights
    as the 2-frame passes -> conv1/conv2 pass-2 matmul cycles halve
  - dynamic-filter products run on DVE in 2x fp16 mode: the di=1 row
    window is served by a separate host copy (xts) so every slice start
    is 4B-aligned
  - only the CENTER (dj=1) pixel-partition x copy is loaded; the dj
    column shift moves to the kernel side: kt2 is partition-shifted by
    +-1 via SBUF-SBUF DMA (2KB/partition vs 40KB for x copies) and the
    PE accumulate uses shifted identity matrices (eye(k=+-1)).  The
    edge-replication terms (q=0 dj=0, q=127 dj=2) multiply the same x
    element as the dj=1 term, so they fold into the dj=1 kernel's edge
    values with one tiny DVE add per edge per pass
  - no tree reduction / pass sums: product pairs (same dj) are added
    once on DVE, then PE matmuls accumulate each pair tile into a
    persistent 4-bank PSUM accumulator, interleaved with the next
    pass's conv matmuls (PSUM: 2 conv1 + 2 conv2 + 4 acc banks = 8)
  - normalization term c*S as in v1 (U chain: sums on DVE in the
    pre-product idle window, shifts/box-sums on GpSimd)
  - output: acc -> fp16 -> 16 DMA-xbar transposes -> DRAM (host casts
    to fp32); no PE transposes, no fp32 identity
  - startup: w1 + pass-0 conv input bands issued first on the sync
    HWDGE ring; all other constants + pass-1/2 inputs on the act ring
"""

import numpy as np

DIM = 64
T = 5
H = 128
W = 128
SLAB = 32          # output rows per core
NCORES = 8
GH = 36            # conv grid rows, passes 0/1: slab + 2*2 halo
GH2 = 20           # conv grid rows, pass 2 halves: 16 + 2*2 halo
GW = 130           # conv grid cols: W + 2
FR = 34            # filter rows: slab + 2 halo
NPASS = 3

_PROGRAM_CACHE = {}

C1_CHUNKS = [(1 + 4 * i, 4) for i in range(8)] + [(33, 2)]
C2_CHUNKS = [(2 + 4 * i, 4) for i in range(8)]
C1_CHUNKS2 = [(1 + 4 * i, 4) for i in range(4)] + [(17, 2)]
C2_CHUNKS2 = [(2 + 4 * i, 4) for i in range(4)]


def _build_program():
    import concourse.bacc as bacc
    import concourse.mybir as mybir
    from concourse.tile import TileContext

    f32 = mybir.dt.float32
    f16 = mybir.dt.float16
    u16 = mybir.dt.uint16
    Act = mybir.ActivationFunctionType
    Alu = mybir.AluOpType
    Ax = mybir.AxisListType

    nc = bacc.Bacc("TRN2", debug=False)

    xc01_d = nc.dram_tensor("xc01", [2, 128, GH, GW], f16, kind="ExternalInput").ap()
    xc2_d = nc.dram_tensor("xc2", [128, GH2, GW], f16, kind="ExternalInput").ap()
    xt_d = nc.dram_tensor("xt", [W, T, DIM, FR], f16, kind="ExternalInput").ap()
    xts_d = nc.dram_tensor("xts", [W, T, DIM, SLAB], f16, kind="ExternalInput").ap()
    sm_d = nc.dram_tensor("sm", [128, 2, 128], f16, kind="ExternalInput").ap()
    em_d = nc.dram_tensor("em", [128, 2], f32, kind="ExternalInput").ap()
    w1_d = nc.dram_tensor("w1", [128, 9, 128], f16, kind="ExternalInput").ap()
    w2_d = nc.dram_tensor("w2", [128, 9, 18], f16, kind="ExternalInput").ap()
    b1_d = nc.dram_tensor("b1r", [128, 1], f32, kind="ExternalInput").ap()
    b2_d = nc.dram_tensor("b2r", [18, 1], f32, kind="ExternalInput").ap()
    ym_d = nc.dram_tensor("ym", [128, 2], f32, kind="ExternalInput").ap()
    ym2_d = nc.dram_tensor("ym2", [128, 2], f32, kind="ExternalInput").ap()
    idf_d = nc.dram_tensor("idf", [128, 128], f16, kind="ExternalInput").ap()
    # out[b_, a, q] = acc[q, 128*a + b_]; the host unscrambles (c r) =
    # 128*a + b_ back to [c, r] (one contiguous DMA instead of 16)
    out_d = nc.dram_tensor("out", [128, 16, W], f16, kind="ExternalOutput").ap()

    with TileContext(nc) as tc:
        with (
            tc.tile_pool(name="consts", bufs=1) as cpool,
            tc.tile_pool(name="xtp", bufs=1) as xtp,
            tc.tile_pool(name="xcp", bufs=2) as xcp,
            tc.tile_pool(name="xc2p", bufs=1) as xc2p,
            tc.tile_pool(name="yp", bufs=3) as yp,
            tc.tile_pool(name="ksh", bufs=2) as kshp,
            tc.tile_pool(name="kst", bufs=1) as kstp,
            tc.tile_pool(name="kta", bufs=1) as ktap,
            tc.tile_pool(name="ktp", bufs=1) as ktp,
            tc.tile_pool(name="up", bufs=1) as up,
            tc.tile_pool(name="tp", bufs=8) as tp,
            tc.tile_pool(name="obp", bufs=1) as obp,
        ):
            # ---- startup DMAs: sync ring carries only what gates the ----
            # ---- first conv1 matmuls (w1 + pass-0 input bands)        ----
            w1_sb = cpool.tile([128, 9, 128], f16)
            nc.sync.dma_start(out=w1_sb, in_=w1_d)

            def load_xc01(p, eng):
                t = xcp.tile([128, GH, GW], f16, tag="xc")
                for r0b, r1b in ((0, 8), (8, 16), (16, 24), (24, 32), (32, 36)):
                    eng.dma_start(out=t[:, r0b:r1b], in_=xc01_d[p, :, r0b:r1b])
                return t

            xc_p0 = load_xc01(0, nc.sync)

            # filter inputs on the sync HWDGE ring AFTER the pass-0 conv
            # bands: per-ring FIFO means the bands drain at full
            # bandwidth first (on the SWDGE ring their huge descriptors
            # monopolized the SDMA engines and stalled conv1 pass 0)
            xt1 = xtp.tile([W, T, DIM, FR], f16, name="xt1")
            xts1 = xtp.tile([W, T, DIM, SLAB], f16, name="xts1")
            nc.sync.dma_start(out=xt1, in_=xt_d)
            nc.sync.dma_start(out=xts1, in_=xts_d)

            # act HWDGE ring: everything else, in need-order
            b1_sb = cpool.tile([128, 1], f32)
            nc.scalar.dma_start(out=b1_sb, in_=b1_d)
            w2_sb = cpool.tile([128, 9, 18], f16)
            nc.scalar.dma_start(out=w2_sb, in_=w2_d)
            b2_sb = cpool.tile([18, 1], f32)
            nc.scalar.dma_start(out=b2_sb, in_=b2_d)
            ym_sb = cpool.tile([128, 2], f32)
            nc.scalar.dma_start(out=ym_sb, in_=ym_d)
            ym2_sb = cpool.tile([128, 2], f32)
            nc.scalar.dma_start(out=ym2_sb, in_=ym2_d)
            idf_sb = cpool.tile([128, 128], f16)
            nc.scalar.dma_start(out=idf_sb, in_=idf_d)
            sm_sb = cpool.tile([128, 2, 128], f16)
            nc.scalar.dma_start(out=sm_sb, in_=sm_d)
            em_sb = cpool.tile([128, 2], f32)
            nc.scalar.dma_start(out=em_sb, in_=em_d)
            # pass-1/2 conv inputs also on the sync ring: DMAs on the act
            # ring would block the conv1 Prelu drains behind their
            # completions (act-queue FIFO) and stall conv1 on PSUM reuse
            xc_p1 = load_xc01(1, nc.sync)
            xc_p2 = xc2p.tile([128, GH2, GW], f16)
            for r0b, r1b in ((0, 8), (8, 16), (16, 20)):
                nc.sync.dma_start(out=xc_p2[:, r0b:r1b], in_=xc2_d[:, r0b:r1b])

            # conv2 -> kernel staging (ti on partitions)
            ker_st = kstp.tile([32, SLAB, W], f16)
            nc.gpsimd.memset(ker_st.bitcast(u16), 0)

            # y tiles pre-allocated; edge cols zeroed up front on gpsimd
            y_t = [yp.tile([128, GH, GW], f16, name=f"y{p}", tag="y")
                   for p in range(3)]
            for p in range(3):
                nr = 34 if p < 2 else 18
                nc.gpsimd.memset(y_t[p][:, 1:1 + nr, 0:1].bitcast(u16), 0)
                nc.gpsimd.memset(y_t[p][:, 1:1 + nr, 129:130].bitcast(u16), 0)

            kt2 = [ktp.tile([W, 32, SLAB], f16, name=f"kt2_{p}")
                   for p in range(NPASS)]

            # U chain part 1 on DVE: fills the idle window before the
            # first products (xt arrives ~35us, first kernels ~41us)
            u_c = up.tile([W, DIM, FR], f16, name="u_c")
            u_m1 = up.tile([W, DIM, FR], f16, name="u_m1")
            u_p1 = up.tile([W, DIM, FR], f16, name="u_p1")
            sv = up.tile([W, DIM, SLAB], f16, name="sv")
            nc.vector.tensor_tensor(u_c, xt1[:, 0], xt1[:, 1], Alu.add)
            for t_i in (2, 3, 4):
                nc.vector.tensor_tensor(u_c, u_c, xt1[:, t_i], Alu.add)

            def emit_u_part2():
                # partition-shifted copies via DMA (engines are lockstep;
                # only DMA can shift partitions); edges replicate
                nc.gpsimd.dma_start(out=u_m1[1:128], in_=u_c[0:127])
                nc.gpsimd.dma_start(out=u_m1[0:1], in_=u_c[0:1])
                nc.gpsimd.dma_start(out=u_p1[0:127], in_=u_c[1:128])
                nc.gpsimd.dma_start(out=u_p1[127:128], in_=u_c[127:128])
                nc.gpsimd.tensor_tensor(u_m1, u_c, u_m1, Alu.add)
                nc.gpsimd.tensor_tensor(u_m1, u_m1, u_p1, Alu.add)
                nc.gpsimd.tensor_tensor(sv, u_m1[:, :, 0:SLAB],
                                        u_m1[:, :, 1:SLAB + 1], Alu.add)
                nc.gpsimd.tensor_tensor(sv, sv, u_m1[:, :, 2:SLAB + 2], Alu.add)

            r_p = [ktp.tile([W, SLAB], f32, name=f"r{p}") for p in range(NPASS)]

            with (
                tc.tile_pool(name="ps1", bufs=2, space="PSUM") as ps1p,
                tc.tile_pool(name="ps2", bufs=2, space="PSUM") as ps2p,
                tc.tile_pool(name="acc", bufs=1, space="PSUM") as accp,
            ):
                acc = accp.tile([W, DIM * SLAB], f32)
                pending = []          # (tile, dj) awaiting PE accumulate
                acc_state = {"first": True}

                def acc_mm(tile, dj, last):
                    # dj=1: plain identity; dj=0/2: shifted identity
                    # applies the +-1 pixel-column shift of the patches
                    lhs = (sm_sb[:, 0, :], idf_sb, sm_sb[:, 1, :])[dj]
                    fl = tile.rearrange("q c r -> q (c r)")
                    for cc in range(4):
                        sl = slice(512 * cc, 512 * (cc + 1))
                        nc.tensor.matmul(acc[:, sl], lhsT=lhs, rhs=fl[:, sl],
                                         start=acc_state["first"], stop=last)
                    acc_state["first"] = False

                def drain_acc(n):
                    for _ in range(min(n, len(pending))):
                        tile, dj = pending.pop(0)
                        acc_mm(tile, dj, False)

                def final_drain():
                    while len(pending) > 1:
                        drain_acc(1)
                    tile, dj = pending.pop(0)
                    acc_mm(tile, dj, True)

                for p in range(NPASS):
                    xc_f = (xc_p0, xc_p1, xc_p2)[p]
                    y_f = y_t[p]
                    c1 = C1_CHUNKS if p < 2 else C1_CHUNKS2
                    c2 = C2_CHUNKS if p < 2 else C2_CHUNKS2

                    # conv1 + leaky relu (single Prelu drain per chunk)
                    for ci, (g0, nr) in enumerate(c1):
                        ps = ps1p.tile([128, 4, W], f32, tag="ps1")
                        for idx in range(9):
                            di, dj = divmod(idx, 3)
                            rhs = xc_f[:, g0 + di - 1:g0 + di - 1 + nr,
                                       dj:dj + W]
                            nc.tensor.matmul(
                                ps[:, :nr, :], lhsT=w1_sb[:, idx, :], rhs=rhs,
                                start=(idx == 0), stop=(idx == 8))
                        nc.scalar.activation(y_f[:, g0:g0 + nr, 1:129],
                                             ps[:, :nr], Act.Prelu,
                                             bias=b1_sb, scale=1.0, alpha=0.2)
                        # interleave pending accumulates from the previous
                        # pass (PE slightly ahead of DVE -> no stall)
                        if p > 0 and ci >= 2 and ci % 2 == 0:
                            drain_acc(1)

                    # conv2 zero-pads rows outside the image: kill y halo
                    ymm = ym_sb if p < 2 else ym2_sb
                    hrow = 34 if p < 2 else 18
                    nc.scalar.activation(y_f[:, 1:2, 1:129],
                                         y_f[:, 1:2, 1:129],
                                         Act.Copy, scale=ymm[:, 0:1])
                    nc.scalar.activation(y_f[:, hrow:hrow + 1, 1:129],
                                         y_f[:, hrow:hrow + 1, 1:129],
                                         Act.Copy, scale=ymm[:, 1:2])

                    # conv2 -> ker_st[ti, r, q]
                    for ci, (g0, nr) in enumerate(c2):
                        ps2 = ps2p.tile([18, 4, W], f32, tag="ps2")
                        for idx in range(9):
                            di, dj = divmod(idx, 3)
                            rhs = y_f[:, g0 + di - 1:g0 + di - 1 + nr,
                                      dj:dj + W]
                            nc.tensor.matmul(
                                ps2[:, :nr, :], lhsT=w2_sb[:, idx, :], rhs=rhs,
                                start=(idx == 0), stop=(idx == 8))
                        nc.scalar.activation(ker_st[0:18, g0 - 2:g0 - 2 + nr, :],
                                             ps2[:, :nr], Act.Identity,
                                             bias=b2_sb, scale=1.0)
                        if p > 0 and ci % 2 == 1:
                            drain_acc(1)

                    # DMA xbar transpose to pixel partitions, then repack
                    # on gpsimd to kt2[q, ti, r] (r innermost for the
                    # product broadcast)
                    if p < 2:
                        ktA = ktap.tile([W, SLAB, 32], f16, tag="ktA")
                        nc.sync.dma_start_transpose(
                            out=ktA, in_=ker_st.rearrange("ti r q -> ti (r q)"))
                        nc.gpsimd.tensor_copy(
                            kt2[p], ktA.rearrange("q r ti -> q ti r"))
                    else:
                        ktA = ktap.tile([W, 16, 32], f16, tag="ktA2")
                        nc.sync.dma_start_transpose(
                            out=ktA,
                            in_=ker_st[:, 0:16, :].rearrange("ti r q -> ti (r q)"))
                        # halves: taps 0-8 = kernel rows 0-15, taps 9-17 =
                        # kernel rows 16-31
                        nc.gpsimd.tensor_copy(
                            kt2[2][:, 0:9, 0:16],
                            ktA[:, :, 0:9].rearrange("q r ti -> q ti r"))
                        nc.gpsimd.tensor_copy(
                            kt2[2][:, 0:9, 16:32],
                            ktA[:, :, 9:18].rearrange("q r ti -> q ti r"))

                    if p == 0:
                        emit_u_part2()

                    # per-pass kernel sum for the normalization coefficient
                    # (must read the PRE-merge kernel values)
                    nt = 18 if p < 2 else 9
                    nc.vector.tensor_reduce(
                        r_p[p], kt2[p].rearrange("q ti r -> q r ti")[:, :, 0:nt],
                        axis=Ax.X, op=Alu.add)

                    # fold the edge-replicated dj=0 (q=0) / dj=2 (q=127)
                    # terms into the dj=1 kernel: they multiply the same x
                    # element as the dj=1 term at that column.  Engines
                    # can't start mid-partition, so mask with a per-
                    # partition one-hot: dj1 += onehot(edge) * dj_edge
                    ev = kt2[p][:, 0:nt, :].rearrange("q (a b) r -> q a b r",
                                                      b=3)
                    nc.vector.scalar_tensor_tensor(
                        ev[:, :, 1, :], ev[:, :, 0, :], em_sb[:, 0:1],
                        ev[:, :, 1, :], Alu.mult, Alu.add)
                    nc.vector.scalar_tensor_tensor(
                        ev[:, :, 1, :], ev[:, :, 2, :], em_sb[:, 1:2],
                        ev[:, :, 1, :], Alu.mult, Alu.add)

                    # partition-shifted kernel copies for dj=0 / dj=2
                    # (products run at the source pixel; the PE accumulate
                    # shifts them into place with eye(k=+-1))
                    ktp_t = kshp.tile([W, 32, SLAB], f16, tag="kp")
                    ktm_t = kshp.tile([W, 32, SLAB], f16, tag="km")
                    nc.gpsimd.dma_start(out=ktp_t[0:127], in_=kt2[p][1:128])
                    nc.gpsimd.dma_start(out=ktp_t[127:128], in_=kt2[p][127:128])
                    nc.gpsimd.dma_start(out=ktm_t[1:128], in_=kt2[p][0:127])
                    nc.gpsimd.dma_start(out=ktm_t[0:1], in_=kt2[p][0:1])

                    # dynamic-filter products: pairs (same dj) -> one DVE
                    # add -> PE accumulate (drained interleaved with conv)
                    if p < 2:
                        groups = [[(2 * p + fi, fi * 9 + 3 * di + dj, di, dj)
                                   for fi in (0, 1)]
                                  for dj in (1, 0, 2) for di in range(3)]
                    else:
                        groups = []
                        for dj in (1, 0, 2):
                            terms = [(4, 3 * di + dj, di, dj)
                                     for di in range(3)]
                            groups += [terms[0:2], terms[2:3]]
                    for g in groups:
                        prods = []
                        for (f, ti, di, dj) in g:
                            src = (ktp_t, kt2[p], ktm_t)[dj]
                            kb = src[:, ti, :].unsqueeze(1)\
                                .broadcast_to((W, DIM, SLAB))
                            if di == 1:
                                xs = xts1[:, f, :, :]
                            else:
                                xs = xt1[:, f, :, di:di + SLAB]
                            prod = tp.tile([W, DIM, SLAB], f16, tag="ts")
                            nc.vector.tensor_tensor(prod, xs, kb, Alu.mult)
                            prods.append(prod)
                        if len(prods) == 2:
                            nc.vector.tensor_tensor(prods[0], prods[0],
                                                    prods[1], Alu.add)
                        pending.append((prods[0], g[0][3]))

                # --- normalization: c = 1/45 - mean(ker); out += c * S ---
                nc.vector.tensor_tensor(r_p[0], r_p[0], r_p[1], Alu.add)
                nc.vector.tensor_tensor(r_p[0], r_p[0], r_p[2], Alu.add)
                c_sb = ktp.tile([W, SLAB], f32, name="c_sb")
                nc.vector.tensor_scalar(c_sb, r_p[0], -1.0 / 45.0, 1.0 / 45.0,
                                        Alu.mult, Alu.add)
                c_bf = ktp.tile([W, SLAB], f16, name="c_bf")
                nc.vector.tensor_copy(c_bf, c_sb)
                cs_prod = tp.tile([W, DIM, SLAB], f16, tag="ts")
                cb = c_bf.unsqueeze(1).broadcast_to((W, DIM, SLAB))
                nc.vector.tensor_tensor(cs_prod, sv, cb, Alu.mult)
                pending.append((cs_prod, 1))

                final_drain()

                # drain acc -> fp16, then DMA-xbar transposes to DRAM
                acc_sb = ktp.tile([W, DIM * SLAB], f16, name="acc_sb")
                for cc in range(4):
                    sl = slice(512 * cc, 512 * (cc + 1))
                    nc.scalar.activation(acc_sb[:, sl], acc[:, sl],
                                         Act.Copy, scale=1.0)
                ob = obp.tile([128, 16, W], f16, tag="ob")
                nc.sync.dma_start_transpose(out=ob, in_=acc_sb)
                nc.sync.dma_start(out=out_d, in_=ob)

    return nc


def _get_program():
    if "nc" not in _PROGRAM_CACHE:
        nc = _build_program()
        nc.finalize()
        _PROGRAM_CACHE["nc"] = nc
    return _PROGRAM_CACHE["nc"]


def _host_prep(x, w1, b1, w2, b2):
    """Build the 8 per-core input maps from full inputs."""
    x = np.asarray(x, dtype=np.float32)
    w1 = np.asarray(w1, dtype=np.float32)
    b1 = np.asarray(b1, dtype=np.float32)
    w2 = np.asarray(w2, dtype=np.float32)
    b2 = np.asarray(b2, dtype=np.float32)
    f16 = np.float16

    # block-diagonal packed weights: passes 0/1 = 2 frames, pass 2 = the
    # two spatial halves of frame 4 -> identical weight matrices
    w1t = w1.transpose(1, 2, 3, 0).reshape(DIM, 9, DIM)   # [ci, tap, o]
    w2t = w2.transpose(1, 2, 3, 0).reshape(DIM, 9, 9)
    w1a = np.zeros((128, 9, 128), np.float32)
    w1a[0:64, :, 0:64] = w1t
    w1a[64:128, :, 64:128] = w1t
    w2a = np.zeros((128, 9, 18), np.float32)
    w2a[0:64, :, 0:9] = w2t
    w2a[64:128, :, 9:18] = w2t

    b1r = np.concatenate([b1, b1]).reshape(128, 1).astype(np.float32)
    b2r = np.concatenate([b2, b2]).reshape(18, 1).astype(np.float32)
    idf = np.eye(128, dtype=f16)
    w1a = w1a.astype(f16)
    w2a = w2a.astype(f16)

    in_maps = []
    for core in range(NCORES):
        b, s = divmod(core, 4)
        r0 = s * SLAB
        # passes 0/1 conv input: frames (2p, 2p+1) on the partition
        # halves, x rows r0-2 .. r0+33 zero padded, cols -1..128 zero
        xc01 = np.zeros((2, 128, GH, GW), np.float32)
        lo = max(0, r0 - 2)
        hi = min(H, r0 + 34)
        for p in range(2):
            for f in range(2):
                t = 2 * p + f
                xc01[p, f * 64:(f + 1) * 64,
                     lo - (r0 - 2):hi - (r0 - 2), 1:129] = x[b, :, t, lo:hi, :]
        # pass 2: frame 4 split into two 16-row halves on the partition
        # halves (plus conv halo)
        xc2 = np.zeros((128, GH2, GW), np.float32)
        for h2 in range(2):
            bx = r0 - 2 if h2 == 0 else r0 + 14
            lo2 = max(0, bx)
            hi2 = min(H, bx + GH2)
            xc2[h2 * 64:(h2 + 1) * 64, lo2 - bx:hi2 - bx, 1:129] = \
                x[b, :, 4, lo2:hi2, :]
        # filter input, pixel-partition, center (dj=1) copy only; xts =
        # the r0-based row window so di=1 product slices start 4B-aligned
        rows = np.clip(np.arange(r0 - 1, r0 + 33), 0, H - 1)
        xt = x[b][:, :, rows, :].transpose(3, 1, 0, 2)          # (w,t,c,34)
        xts = x[b][:, :, r0:r0 + 32, :].transpose(3, 1, 0, 2)   # (w,t,c,32)
        # shifted identities for the dj=0/dj=2 accumulates
        sm = np.zeros((128, 2, 128), np.float32)
        sm[0:127, 0, :] = np.eye(128, dtype=np.float32)[1:128]   # m = p+1
        sm[1:128, 1, :] = np.eye(128, dtype=np.float32)[0:127]   # m = p-1
        em = np.zeros((128, 2), np.float32)
        em[0, 0] = 1.0      # q=0 edge (dj=0 term folds into dj=1)
        em[127, 1] = 1.0    # q=127 edge (dj=2 term folds into dj=1)
        # conv2 zero-pad masks for y rows outside the image
        ym = np.ones((128, 2), np.float32)
        if s == 0:
            ym[:, 0] = 0.0
        if s == 3:
            ym[:, 1] = 0.0
        ym2 = np.ones((128, 2), np.float32)
        if s == 0:
            ym2[0:64, 0] = 0.0
        if s == 3:
            ym2[64:128, 1] = 0.0
        in_maps.append({
            "xc01": xc01.astype(f16), "xc2": xc2.astype(f16),
            "xt": xt.astype(f16), "xts": xts.astype(f16),
            "w1": w1a, "w2": w2a, "b1r": b1r, "b2r": b2r,
            "ym": ym, "ym2": ym2, "idf": idf, "sm": sm.astype(f16),
            "em": em,
        })
    return in_maps


def kernel(x, w1, b1, w2, b2):
    from concourse.bass_utils import run_bass_kernel_spmd

    nc = _get_program()
    in_maps = _host_prep(x, w1, b1, w2, b2)
    res = run_bass_kernel_spmd(nc, in_maps, list(range(NCORES)))
    out = np.zeros((2, DIM, H, W), dtype=np.float32)
    for core in range(NCORES):
        b, s = divmod(core, 4)
        # device layout: o[b_, a, q] = result[(c r) = 128*a + b_, q]
        o = res.results[core]["out"].astype(np.float32)
        o = o.transpose(1, 0, 2).reshape(DIM, SLAB, W)
        out[b, :, s * SLAB:(s + 1) * SLAB, :] = o
    return out


# revision 30
# speedup vs baseline: 1.1020x; 1.0085x over previous
"""Trainium2 Bass kernel for nn_DynamicFiltering (v2).

Computation (per batch b):
  y  = LeakyReLU(conv2d(x_t, w1, b1), 0.2)        per frame t
  ker = conv2d(y, w2, b2)                          (t, 9, h, w)
  ker = ker - mean_K(ker) + 1/45                   per-pixel over K = 45
  out[c,h,w] = sum_{t,k1,k2} x_edge[c,t,h+k1-1,w+k2-1] * ker[t,k1,k2][h,w]

Sharding: 8 cores = 2 batches x 4 H-slabs of 32 rows.

Structure (v2, vs the 234us bf16 baseline):
  - all 16-bit data is fp16 (same speed as bf16, ~8x less quant error)
  - conv1 leaky relu is a single Act Prelu(alpha=0.2) drain (verified on
    HW: Prelu honors alpha, Lrelu hardcodes 0.01) -> GpSimd fully freed
  - pass 2 (lone frame 4) is spatially halved: both partition halves
    carry frame-4 channels over half the rows, same block-diag weights
    as the 2-frame passes -> conv1/conv2 pass-2 matmul cycles halve
  - dynamic-filter products run on DVE in 2x fp16 mode: the di=1 row
    window is served by a separate host copy (xts) so every slice start
    is 4B-aligned
  - only the CENTER (dj=1) pixel-partition x copy is loaded; the dj
    column shift moves to the kernel side: kt2 is partition-shifted by
    +-1 via SBUF-SBUF DMA (2KB/partition vs 40KB for x copies) and the
    PE accumulate uses shifted identity matrices (eye(k=+-1)).  The
    edge-replication terms (q=0 dj=0, q=127 dj=2) multiply the same x
    element as the dj=1 term, so they fold into the dj=1 kernel's edge
    values with one tiny DVE add per edge per pass
  - no tree reduction / pass sums: product pairs (same dj) are added
    once on DVE, then PE matmuls accumulate each pair tile into a
    persistent 4-bank PSUM accumulator, interleaved with the next
    pass's conv matmuls (PSUM: 2 conv1 + 2 conv2 + 4 acc banks = 8)
  - normalization term c*S as in v1 (U chain: sums on DVE in the
    pre-product idle window, shifts/box-sums on GpSimd)
  - output: acc -> fp16 -> 16 DMA-xbar transposes -> DRAM (host casts
    to fp32); no PE transposes, no fp32 identity
  - startup: w1 + pass-0 conv input bands issued first on the sync
    HWDGE ring; all other constants + pass-1/2 inputs on the act ring
"""

import numpy as np

DIM = 64
T = 5
H = 128
W = 128
SLAB = 32          # output rows per core
NCORES = 8
GH = 36            # conv grid rows, passes 0/1: slab + 2*2 halo
GH2 = 20           # conv grid rows, pass 2 halves: 16 + 2*2 halo
GW = 130           # conv grid cols: W + 2
FR = 34            # filter rows: slab + 2 halo
NPASS = 3

_PROGRAM_CACHE = {}

C1_CHUNKS = [(1 + 4 * i, 4) for i in range(8)] + [(33, 2)]
C2_CHUNKS = [(2 + 4 * i, 4) for i in range(8)]
C1_CHUNKS2 = [(1 + 4 * i, 4) for i in range(4)] + [(17, 2)]
C2_CHUNKS2 = [(2 + 4 * i, 4) for i in range(4)]


def _build_program():
    import concourse.bacc as bacc
    import concourse.mybir as mybir
    from concourse.tile import TileContext

    f32 = mybir.dt.float32
    f16 = mybir.dt.float16
    u16 = mybir.dt.uint16
    Act = mybir.ActivationFunctionType
    Alu = mybir.AluOpType
    Ax = mybir.AxisListType

    nc = bacc.Bacc("TRN2", debug=False)

    xc01_d = nc.dram_tensor("xc01", [2, 128, GH, GW], f16, kind="ExternalInput").ap()
    xc2_d = nc.dram_tensor("xc2", [128, GH2, GW], f16, kind="ExternalInput").ap()
    xt_d = nc.dram_tensor("xt", [W, T, DIM, FR], f16, kind="ExternalInput").ap()
    xts_d = nc.dram_tensor("xts", [W, T, DIM, SLAB], f16, kind="ExternalInput").ap()
    sm_d = nc.dram_tensor("sm", [128, 2, 128], f16, kind="ExternalInput").ap()
    em_d = nc.dram_tensor("em", [128, 2], f32, kind="ExternalInput").ap()
    w1_d = nc.dram_tensor("w1", [128, 9, 128], f16, kind="ExternalInput").ap()
    w2_d = nc.dram_tensor("w2", [128, 9, 18], f16, kind="ExternalInput").ap()
    b1_d = nc.dram_tensor("b1r", [128, 1], f32, kind="ExternalInput").ap()
    b2_d = nc.dram_tensor("b2r", [18, 1], f32, kind="ExternalInput").ap()
    ym_d = nc.dram_tensor("ym", [128, 2], f32, kind="ExternalInput").ap()
    ym2_d = nc.dram_tensor("ym2", [128, 2], f32, kind="ExternalInput").ap()
    idf_d = nc.dram_tensor("idf", [128, 128], f16, kind="ExternalInput").ap()
    # out[b_, a, q] = acc[q, 128*a + b_]; the host unscrambles (c r) =
    # 128*a + b_ back to [c, r] (one contiguous DMA instead of 16)
    out_d = nc.dram_tensor("out", [128, 16, W], f16, kind="ExternalOutput").ap()

    with TileContext(nc) as tc:
        with (
            tc.tile_pool(name="consts", bufs=1) as cpool,
            tc.tile_pool(name="xtp", bufs=1) as xtp,
            tc.tile_pool(name="xcp", bufs=2) as xcp,
            tc.tile_pool(name="xc2p", bufs=1) as xc2p,
            tc.tile_pool(name="yp", bufs=3) as yp,
            tc.tile_pool(name="ksh", bufs=2) as kshp,
            tc.tile_pool(name="kst", bufs=1) as kstp,
            tc.tile_pool(name="kta", bufs=1) as ktap,
            tc.tile_pool(name="ktp", bufs=1) as ktp,
            tc.tile_pool(name="up", bufs=1) as up,
            tc.tile_pool(name="tp", bufs=8) as tp,
            tc.tile_pool(name="obp", bufs=1) as obp,
        ):
            # ---- startup DMAs: sync ring carries only what gates the ----
            # ---- first conv1 matmuls (w1 + pass-0 input bands)        ----
            w1_sb = cpool.tile([128, 9, 128], f16)
            nc.sync.dma_start(out=w1_sb, in_=w1_d)

            def load_xc01(p, eng):
                t = xcp.tile([128, GH, GW], f16, tag="xc")
                for r0b, r1b in ((0, 8), (8, 16), (16, 24), (24, 32), (32, 36)):
                    eng.dma_start(out=t[:, r0b:r1b], in_=xc01_d[p, :, r0b:r1b])
                return t

            xc_p0 = load_xc01(0, nc.sync)

            # filter inputs on the sync HWDGE ring AFTER the pass-0 conv
            # bands: per-ring FIFO means the bands drain at full
            # bandwidth first (on the SWDGE ring their huge descriptors
            # monopolized the SDMA engines and stalled conv1 pass 0)
            xt1 = xtp.tile([W, T, DIM, FR], f16, name="xt1")
            xts1 = xtp.tile([W, T, DIM, SLAB], f16, name="xts1")
            nc.sync.dma_start(out=xt1, in_=xt_d)
            nc.sync.dma_start(out=xts1, in_=xts_d)

            # act HWDGE ring: everything else, in need-order
            b1_sb = cpool.tile([128, 1], f32)
            nc.scalar.dma_start(out=b1_sb, in_=b1_d)
            w2_sb = cpool.tile([128, 9, 18], f16)
            nc.scalar.dma_start(out=w2_sb, in_=w2_d)
            b2_sb = cpool.tile([18, 1], f32)
            nc.scalar.dma_start(out=b2_sb, in_=b2_d)
            ym_sb = cpool.tile([128, 2], f32)
            nc.scalar.dma_start(out=ym_sb, in_=ym_d)
            ym2_sb = cpool.tile([128, 2], f32)
            nc.scalar.dma_start(out=ym2_sb, in_=ym2_d)
            idf_sb = cpool.tile([128, 128], f16)
            nc.scalar.dma_start(out=idf_sb, in_=idf_d)
            sm_sb = cpool.tile([128, 2, 128], f16)
            nc.scalar.dma_start(out=sm_sb, in_=sm_d)
            em_sb = cpool.tile([128, 2], f32)
            nc.scalar.dma_start(out=em_sb, in_=em_d)
            # pass-1/2 conv inputs also on the sync ring: DMAs on the act
            # ring would block the conv1 Prelu drains behind their
            # completions (act-queue FIFO) and stall conv1 on PSUM reuse
            xc_p1 = load_xc01(1, nc.sync)
            xc_p2 = xc2p.tile([128, GH2, GW], f16)
            for r0b, r1b in ((0, 8), (8, 16), (16, 20)):
                nc.sync.dma_start(out=xc_p2[:, r0b:r1b], in_=xc2_d[:, r0b:r1b])

            # conv2 -> kernel staging (ti on partitions)
            ker_st = kstp.tile([32, SLAB, W], f16)
            nc.gpsimd.memset(ker_st.bitcast(u16), 0)

            # y tiles pre-allocated; edge cols zeroed up front on gpsimd
            y_t = [yp.tile([128, GH, GW], f16, name=f"y{p}", tag="y")
                   for p in range(3)]
            for p in range(3):
                nr = 34 if p < 2 else 18
                nc.gpsimd.memset(y_t[p][:, 1:1 + nr, 0:1].bitcast(u16), 0)
                nc.gpsimd.memset(y_t[p][:, 1:1 + nr, 129:130].bitcast(u16), 0)

            kt2 = [ktp.tile([W, 32, SLAB], f16, name=f"kt2_{p}")
                   for p in range(NPASS)]

            # U chain part 1 on DVE: fills the idle window before the
            # first products (xt arrives ~35us, first kernels ~41us)
            u_c = up.tile([W, DIM, FR], f16, name="u_c")
            u_m1 = up.tile([W, DIM, FR], f16, name="u_m1")
            u_p1 = up.tile([W, DIM, FR], f16, name="u_p1")
            sv = up.tile([W, DIM, SLAB], f16, name="sv")
            nc.vector.tensor_tensor(u_c, xt1[:, 0], xt1[:, 1], Alu.add)
            for t_i in (2, 3, 4):
                nc.vector.tensor_tensor(u_c, u_c, xt1[:, t_i], Alu.add)

            def emit_u_part2():
                # partition-shifted copies via DMA (engines are lockstep;
                # only DMA can shift partitions); edges replicate
                nc.gpsimd.dma_start(out=u_m1[1:128], in_=u_c[0:127])
                nc.gpsimd.dma_start(out=u_m1[0:1], in_=u_c[0:1])
                nc.gpsimd.dma_start(out=u_p1[0:127], in_=u_c[1:128])
                nc.gpsimd.dma_start(out=u_p1[127:128], in_=u_c[127:128])
                nc.gpsimd.tensor_tensor(u_m1, u_c, u_m1, Alu.add)
                nc.gpsimd.tensor_tensor(u_m1, u_m1, u_p1, Alu.add)
                nc.gpsimd.tensor_tensor(sv, u_m1[:, :, 0:SLAB],
                                        u_m1[:, :, 1:SLAB + 1], Alu.add)
                nc.gpsimd.tensor_tensor(sv, sv, u_m1[:, :, 2:SLAB + 2], Alu.add)

            r_p = [ktp.tile([W, SLAB], f32, name=f"r{p}") for p in range(NPASS)]

            with (
                tc.tile_pool(name="ps1", bufs=2, space="PSUM") as ps1p,
                tc.tile_pool(name="ps2", bufs=2, space="PSUM") as ps2p,
                tc.tile_pool(name="acc", bufs=1, space="PSUM") as accp,
            ):
                acc = accp.tile([W, DIM * SLAB], f32)
                pending = []          # (tile, dj) awaiting PE accumulate
                acc_state = {"first": True}

                def acc_mm(tile, dj, last):
                    # dj=1: plain identity; dj=0/2: shifted identity
                    # applies the +-1 pixel-column shift of the patches
                    lhs = (sm_sb[:, 0, :], idf_sb, sm_sb[:, 1, :])[dj]
                    fl = tile.rearrange("q c r -> q (c r)")
                    for cc in range(4):
                        sl = slice(512 * cc, 512 * (cc + 1))
                        nc.tensor.matmul(acc[:, sl], lhsT=lhs, rhs=fl[:, sl],
                                         start=acc_state["first"], stop=last)
                    acc_state["first"] = False

                def drain_acc(n):
                    for _ in range(min(n, len(pending))):
                        tile, dj = pending.pop(0)
                        acc_mm(tile, dj, False)

                def final_drain():
                    while len(pending) > 1:
                        drain_acc(1)
                    tile, dj = pending.pop(0)
                    acc_mm(tile, dj, True)

                for p in range(NPASS):
                    xc_f = (xc_p0, xc_p1, xc_p2)[p]
                    y_f = y_t[p]
                    c1 = C1_CHUNKS if p < 2 else C1_CHUNKS2
                    c2 = C2_CHUNKS if p < 2 else C2_CHUNKS2

                    # conv1 + leaky relu (single Prelu drain per chunk)
                    for ci, (g0, nr) in enumerate(c1):
                        ps = ps1p.tile([128, 4, W], f32, tag="ps1")
                        for idx in range(9):
                            di, dj = divmod(idx, 3)
                            rhs = xc_f[:, g0 + di - 1:g0 + di - 1 + nr,
                                       dj:dj + W]
                            nc.tensor.matmul(
                                ps[:, :nr, :], lhsT=w1_sb[:, idx, :], rhs=rhs,
                                start=(idx == 0), stop=(idx == 8))
                        nc.scalar.activation(y_f[:, g0:g0 + nr, 1:129],
                                             ps[:, :nr], Act.Prelu,
                                             bias=b1_sb, scale=1.0, alpha=0.2)
                        # interleave pending accumulates from the previous
                        # pass (PE slightly ahead of DVE -> no stall)
                        if (p == 1 and ci >= 4 and ci % 2 == 0) or \
                           (p == 2 and ci >= 1):
                            drain_acc(1)

                    # conv2 zero-pads rows outside the image: kill y halo
                    ymm = ym_sb if p < 2 else ym2_sb
                    hrow = 34 if p < 2 else 18
                    nc.scalar.activation(y_f[:, 1:2, 1:129],
                                         y_f[:, 1:2, 1:129],
                                         Act.Copy, scale=ymm[:, 0:1])
                    nc.scalar.activation(y_f[:, hrow:hrow + 1, 1:129],
                                         y_f[:, hrow:hrow + 1, 1:129],
                                         Act.Copy, scale=ymm[:, 1:2])

                    # conv2 -> ker_st[ti, r, q]
                    for ci, (g0, nr) in enumerate(c2):
                        ps2 = ps2p.tile([18, 4, W], f32, tag="ps2")
                        for idx in range(9):
                            di, dj = divmod(idx, 3)
                            rhs = y_f[:, g0 + di - 1:g0 + di - 1 + nr,
                                      dj:dj + W]
                            nc.tensor.matmul(
                                ps2[:, :nr, :], lhsT=w2_sb[:, idx, :], rhs=rhs,
                                start=(idx == 0), stop=(idx == 8))
                        nc.scalar.activation(ker_st[0:18, g0 - 2:g0 - 2 + nr, :],
                                             ps2[:, :nr], Act.Identity,
                                             bias=b2_sb, scale=1.0)
                        if (p == 1 and ci % 2 == 1) or p == 2:
                            drain_acc(1)

                    # DMA xbar transpose to pixel partitions, then repack
                    # on gpsimd to kt2[q, ti, r] (r innermost for the
                    # product broadcast)
                    if p < 2:
                        ktA = ktap.tile([W, SLAB, 32], f16, tag="ktA")
                        nc.sync.dma_start_transpose(
                            out=ktA, in_=ker_st.rearrange("ti r q -> ti (r q)"))
                        nc.gpsimd.tensor_copy(
                            kt2[p], ktA.rearrange("q r ti -> q ti r"))
                    else:
                        ktA = ktap.tile([W, 16, 32], f16, tag="ktA2")
                        nc.sync.dma_start_transpose(
                            out=ktA,
                            in_=ker_st[:, 0:16, :].rearrange("ti r q -> ti (r q)"))
                        # halves: taps 0-8 = kernel rows 0-15, taps 9-17 =
                        # kernel rows 16-31
                        nc.gpsimd.tensor_copy(
                            kt2[2][:, 0:9, 0:16],
                            ktA[:, :, 0:9].rearrange("q r ti -> q ti r"))
                        nc.gpsimd.tensor_copy(
                            kt2[2][:, 0:9, 16:32],
                            ktA[:, :, 9:18].rearrange("q r ti -> q ti r"))

                    # per-pass kernel sum for the normalization coefficient
                    # (must read the PRE-merge kernel values)
                    nt = 18 if p < 2 else 9
                    nc.vector.tensor_reduce(
                        r_p[p], kt2[p].rearrange("q ti r -> q r ti")[:, :, 0:nt],
                        axis=Ax.X, op=Alu.add)

                    # fold the edge-replicated dj=0 (q=0) / dj=2 (q=127)
                    # terms into the dj=1 kernel: they multiply the same x
                    # element as the dj=1 term at that column.  Engines
                    # can't start mid-partition, so mask with a per-
                    # partition one-hot: dj1 += onehot(edge) * dj_edge
                    ev = kt2[p][:, 0:nt, :].rearrange("q (a b) r -> q a b r",
                                                      b=3)
                    nc.vector.scalar_tensor_tensor(
                        ev[:, :, 1, :], ev[:, :, 0, :], em_sb[:, 0:1],
                        ev[:, :, 1, :], Alu.mult, Alu.add)
                    nc.vector.scalar_tensor_tensor(
                        ev[:, :, 1, :], ev[:, :, 2, :], em_sb[:, 1:2],
                        ev[:, :, 1, :], Alu.mult, Alu.add)

                    # partition-shifted kernel copies for dj=0 / dj=2
                    # (products run at the source pixel; the PE accumulate
                    # shifts them into place with eye(k=+-1))
                    ktp_t = kshp.tile([W, 32, SLAB], f16, tag="kp")
                    ktm_t = kshp.tile([W, 32, SLAB], f16, tag="km")
                    nc.gpsimd.dma_start(out=ktp_t[0:127], in_=kt2[p][1:128])
                    nc.gpsimd.dma_start(out=ktp_t[127:128], in_=kt2[p][127:128])
                    nc.gpsimd.dma_start(out=ktm_t[1:128], in_=kt2[p][0:127])
                    nc.gpsimd.dma_start(out=ktm_t[0:1], in_=kt2[p][0:1])

                    # dynamic-filter products: pairs (same dj) -> one DVE
                    # add -> PE accumulate (drained interleaved with conv)
                    if p < 2:
                        groups = [[(2 * p + fi, fi * 9 + 3 * di + dj, di, dj)
                                   for fi in (0, 1)]
                                  for dj in (1, 0, 2) for di in range(3)]
                    else:
                        groups = []
                        for dj in (1, 0, 2):
                            terms = [(4, 3 * di + dj, di, dj)
                                     for di in range(3)]
                            groups += [terms[0:2], terms[2:3]]
                    for gi, g in enumerate(groups):
                        prods = []
                        for (f, ti, di, dj) in g:
                            src = (ktp_t, kt2[p], ktm_t)[dj]
                            kb = src[:, ti, :].unsqueeze(1)\
                                .broadcast_to((W, DIM, SLAB))
                            if di == 1:
                                xs = xts1[:, f, :, :]
                            else:
                                xs = xt1[:, f, :, di:di + SLAB]
                            prod = tp.tile([W, DIM, SLAB], f16, tag="ts")
                            nc.vector.tensor_tensor(prod, xs, kb, Alu.mult)
                            prods.append(prod)
                        if len(prods) == 2:
                            nc.vector.tensor_tensor(prods[0], prods[0],
                                                    prods[1], Alu.add)
                        pending.append((prods[0], g[0][3]))
                        # last pass: no later conv to interleave into, so
                        # drain one older tile per finished group
                        if p == 2 and gi >= 1:
                            drain_acc(1)

                    if p == 0:
                        # U chain part 2 emitted AFTER the pass-0 kt2
                        # shifts: its ~19us of gpsimd tensor ops must not
                        # block the shift DMAs the products wait on
                        emit_u_part2()

                # --- normalization: c = 1/45 - mean(ker); out += c * S ---
                nc.vector.tensor_tensor(r_p[0], r_p[0], r_p[1], Alu.add)
                nc.vector.tensor_tensor(r_p[0], r_p[0], r_p[2], Alu.add)
                c_sb = ktp.tile([W, SLAB], f32, name="c_sb")
                nc.vector.tensor_scalar(c_sb, r_p[0], -1.0 / 45.0, 1.0 / 45.0,
                                        Alu.mult, Alu.add)
                c_bf = ktp.tile([W, SLAB], f16, name="c_bf")
                nc.vector.tensor_copy(c_bf, c_sb)
                cs_prod = tp.tile([W, DIM, SLAB], f16, tag="ts")
                cb = c_bf.unsqueeze(1).broadcast_to((W, DIM, SLAB))
                nc.vector.tensor_tensor(cs_prod, sv, cb, Alu.mult)
                pending.append((cs_prod, 1))

                final_drain()

                # drain acc -> fp16, then DMA-xbar transposes to DRAM
                acc_sb = ktp.tile([W, DIM * SLAB], f16, name="acc_sb")
                for cc in range(4):
                    sl = slice(512 * cc, 512 * (cc + 1))
                    nc.scalar.activation(acc_sb[:, sl], acc[:, sl],
                                         Act.Copy, scale=1.0)
                ob = obp.tile([128, 16, W], f16, tag="ob")
                nc.sync.dma_start_transpose(out=ob, in_=acc_sb)
                nc.sync.dma_start(out=out_d, in_=ob)

    return nc


def _get_program():
    if "nc" not in _PROGRAM_CACHE:
        nc = _build_program()
        nc.finalize()
        _PROGRAM_CACHE["nc"] = nc
    return _PROGRAM_CACHE["nc"]


def _host_prep(x, w1, b1, w2, b2):
    """Build the 8 per-core input maps from full inputs."""
    x = np.asarray(x, dtype=np.float32)
    w1 = np.asarray(w1, dtype=np.float32)
    b1 = np.asarray(b1, dtype=np.float32)
    w2 = np.asarray(w2, dtype=np.float32)
    b2 = np.asarray(b2, dtype=np.float32)
    f16 = np.float16

    # block-diagonal packed weights: passes 0/1 = 2 frames, pass 2 = the
    # two spatial halves of frame 4 -> identical weight matrices
    w1t = w1.transpose(1, 2, 3, 0).reshape(DIM, 9, DIM)   # [ci, tap, o]
    w2t = w2.transpose(1, 2, 3, 0).reshape(DIM, 9, 9)
    w1a = np.zeros((128, 9, 128), np.float32)
    w1a[0:64, :, 0:64] = w1t
    w1a[64:128, :, 64:128] = w1t
    w2a = np.zeros((128, 9, 18), np.float32)
    w2a[0:64, :, 0:9] = w2t
    w2a[64:128, :, 9:18] = w2t

    b1r = np.concatenate([b1, b1]).reshape(128, 1).astype(np.float32)
    b2r = np.concatenate([b2, b2]).reshape(18, 1).astype(np.float32)
    idf = np.eye(128, dtype=f16)
    w1a = w1a.astype(f16)
    w2a = w2a.astype(f16)

    in_maps = []
    for core in range(NCORES):
        b, s = divmod(core, 4)
        r0 = s * SLAB
        # passes 0/1 conv input: frames (2p, 2p+1) on the partition
        # halves, x rows r0-2 .. r0+33 zero padded, cols -1..128 zero
        xc01 = np.zeros((2, 128, GH, GW), np.float32)
        lo = max(0, r0 - 2)
        hi = min(H, r0 + 34)
        for p in range(2):
            for f in range(2):
                t = 2 * p + f
                xc01[p, f * 64:(f + 1) * 64,
                     lo - (r0 - 2):hi - (r0 - 2), 1:129] = x[b, :, t, lo:hi, :]
        # pass 2: frame 4 split into two 16-row halves on the partition
        # halves (plus conv halo)
        xc2 = np.zeros((128, GH2, GW), np.float32)
        for h2 in range(2):
            bx = r0 - 2 if h2 == 0 else r0 + 14
            lo2 = max(0, bx)
            hi2 = min(H, bx + GH2)
            xc2[h2 * 64:(h2 + 1) * 64, lo2 - bx:hi2 - bx, 1:129] = \
                x[b, :, 4, lo2:hi2, :]
        # filter input, pixel-partition, center (dj=1) copy only; xts =
        # the r0-based row window so di=1 product slices start 4B-aligned
        rows = np.clip(np.arange(r0 - 1, r0 + 33), 0, H - 1)
        xt = x[b][:, :, rows, :].transpose(3, 1, 0, 2)          # (w,t,c,34)
        xts = x[b][:, :, r0:r0 + 32, :].transpose(3, 1, 0, 2)   # (w,t,c,32)
        # shifted identities for the dj=0/dj=2 accumulates
        sm = np.zeros((128, 2, 128), np.float32)
        sm[0:127, 0, :] = np.eye(128, dtype=np.float32)[1:128]   # m = p+1
        sm[1:128, 1, :] = np.eye(128, dtype=np.float32)[0:127]   # m = p-1
        em = np.zeros((128, 2), np.float32)
        em[0, 0] = 1.0      # q=0 edge (dj=0 term folds into dj=1)
        em[127, 1] = 1.0    # q=127 edge (dj=2 term folds into dj=1)
        # conv2 zero-pad masks for y rows outside the image
        ym = np.ones((128, 2), np.float32)
        if s == 0:
            ym[:, 0] = 0.0
        if s == 3:
            ym[:, 1] = 0.0
        ym2 = np.ones((128, 2), np.float32)
        if s == 0:
            ym2[0:64, 0] = 0.0
        if s == 3:
            ym2[64:128, 1] = 0.0
        in_maps.append({
            "xc01": xc01.astype(f16), "xc2": xc2.astype(f16),
            "xt": xt.astype(f16), "xts": xts.astype(f16),
            "w1": w1a, "w2": w2a, "b1r": b1r, "b2r": b2r,
            "ym": ym, "ym2": ym2, "idf": idf, "sm": sm.astype(f16),
            "em": em,
        })
    return in_maps


def kernel(x, w1, b1, w2, b2):
    from concourse.bass_utils import run_bass_kernel_spmd

    nc = _get_program()
    in_maps = _host_prep(x, w1, b1, w2, b2)
    res = run_bass_kernel_spmd(nc, in_maps, list(range(NCORES)))
    out = np.zeros((2, DIM, H, W), dtype=np.float32)
    for core in range(NCORES):
        b, s = divmod(core, 4)
        # device layout: o[b_, a, q] = result[(c r) = 128*a + b_, q]
        o = res.results[core]["out"].astype(np.float32)
        o = o.transpose(1, 0, 2).reshape(DIM, SLAB, W)
        out[b, :, s * SLAB:(s + 1) * SLAB, :] = o
    return out
